# revision 1
# baseline (speedup 1.0000x reference)
"""ConditionalRandomField loss kernel for Trainium2 (8 NeuronCores).

Math (per sequence b):
    loss[b] = log_score(gold path) - log_partition
The log_partition forward recursion is computed in exp space:
    f_t[j] = (sum_i E[i,j] * f_{t-1}[i]) * g_t[j]
with E = exp(transitions), g_t = exp(emissions_t), f_0 = exp(start)*g_0,
and periodic per-batch rescaling whose log is accumulated separately:
    log_partition = log(sum_j f_{L-1}[j]*exp(stop[j])) + sum_m log(s_m).

Sharding: data-parallel over batch; core c owns sequences [8c, 8c+8).
Per core the 256-tag state is held as a [128 x (2 jchunk x 8 batch)] bf16
tile; each scan step is 4 PE matmuls (E tiles stationary, bf16 FWL) and one
VE multiply (psum * exp(emissions)).  The gold-path numerator uses
one-hot tag masks: emissions[b,t,tag] via tensor_tensor_reduce against the
staged emission chunks, transitions[tag_t,tag_t+1] via y = Tr^T @ OH matmuls
followed by tensor_tensor_reduce against the shifted one-hot, start/stop via
tiny matmuls.  (HW indirect-DMA only gathers one row per partition, so
per-element gathers are done with masks instead.)

NOTE: mask is all-ones for this problem spec (fill: ones); the kernel
assumes it (the reference's masked branches are identities then).
"""

import numpy as np
from contextlib import ExitStack

import concourse.bass as bass
import concourse.bacc as bacc
import concourse.tile as tile
from concourse import mybir
from concourse.bass_utils import run_bass_kernel_spmd

F32 = mybir.dt.float32


class _Bacc(bacc.Bacc):
    # Keep data waits on the MATMULs so the (data-independent) LDWEIGHTS
    # prefetch during the preceding VE phase instead of stalling the chain.
    # The PE's fg/bg weight buffers interlock LDW-vs-inflight-MM in HW.
    def move_matmul_waits_to_ldweights(self):
        super().move_matmul_waits_to_ldweights()


BF16 = mybir.dt.bfloat16
I32 = mybir.dt.int32

NCORES = 8
B = 64
L = 1024
T = 256
BC = B // NCORES      # sequences per core
PJ = 128              # partition tile of the tag dim
JCN = T // PJ         # = 2 tag chunks
RS = 16               # rescale sampling period (steps)
DEFER = 2             # rescale applied this many steps after sampling
TCH = 128             # emission-load chunk (timesteps per DMA/exp chunk)
CSH = 6.5             # constant log-shift folded into E = exp(transitions - CSH)

AUX_START = T * T          # aux table: [transitions-CSH | start | stop | transitions]
AUX_STOP = T * T + T
AUX_TRRAW = T * T + 2 * T
AUX_N = 2 * T * T + 2 * T


def _sample_steps(length):
    return [t for t in range(1, length) if t % RS == 0 and t + DEFER <= length - 1]


def build_program(length=L, use_gpsimd_oh=False, do_emis=True, do_trans=True, do_ssmm=True):
    """Build the single-core SPMD bass program (each core runs the same
    program on its own batch shard)."""
    assert length % 16 == 0
    l16 = length // 16
    nsamp = len(_sample_steps(length))
    nspad = max(8, ((nsamp + 7) // 8) * 8)

    nc = _Bacc()
    em_t = nc.declare_dram_parameter("em", [BC * length * T, 1], F32, isOutput=False)
    aux_t = nc.declare_dram_parameter("aux", [AUX_N, 1], F32, isOutput=False)
    tags_t = nc.declare_dram_parameter("tags_tb", [length * BC, 1], F32, isOutput=False)
    iota_t = nc.declare_dram_parameter("iota", [128, 1], F32, isOutput=False)
    loss_t = nc.declare_dram_parameter("loss", [BC, 1], F32, isOutput=True)

    def dram_ap(handle, offset, ap):
        full = handle[:]
        return bass.AP(tensor=full.tensor, offset=offset, ap=ap)

    with tile.TileContext(nc) as tc, ExitStack() as ctx:
        const = ctx.enter_context(tc.tile_pool(name="const", bufs=1))
        stage = ctx.enter_context(tc.tile_pool(name="stage", bufs=2))
        gpool = ctx.enter_context(tc.tile_pool(name="gpool", bufs=1))
        fpool = ctx.enter_context(tc.tile_pool(name="fpool", bufs=3))
        vpool = ctx.enter_context(tc.tile_pool(name="vpool", bufs=2))
        ppool = ctx.enter_context(tc.tile_pool(name="ppool", bufs=2, space="PSUM"))
        spool = ctx.enter_context(tc.tile_pool(name="spool", bufs=1, space="PSUM"))
        smallp = ctx.enter_context(tc.tile_pool(name="smallp", bufs=2, space="PSUM"))

        # ---------------- constants / setup ----------------
        # E = exp(transitions) as two [128, 256] bf16 tiles (i-chunk major).
        e_tiles = []
        tr_tiles = []
        for ic in range(JCN):
            eraw = stage.tile([128, T], F32, name=f"eraw{ic}", tag="eraw")
            nc.sync.dma_start(
                out=eraw,
                in_=dram_ap(aux_t, ic * 128 * T, [[T, 128], [1, T]]),
            )
            ebf = const.tile([128, T], BF16, name=f"ebf{ic}")
            # NOTE: the host uploads transitions - CSH in aux, so E here is
            # exp(transitions - CSH): per-step growth ~e^0 keeps running sums
            # inside the ACT Ln range.  The same shift flows into the
            # numerator's transition gathers, so it cancels in the loss.
            nc.scalar.activation(
                out=ebf, in_=eraw, func=mybir.ActivationFunctionType.Exp
            )
            e_tiles.append(ebf)
            # unshifted transitions in bf16 for the gold-score matmuls
            eraw2 = stage.tile([128, T], F32, name=f"eraw2_{ic}", tag="eraw")
            nc.sync.dma_start(
                out=eraw2,
                in_=dram_ap(aux_t, AUX_TRRAW + ic * 128 * T, [[T, 128], [1, T]]),
            )
            trbf = const.tile([128, T], BF16, name=f"trbf{ic}")
            nc.vector.tensor_copy(out=trbf, in_=eraw2)
            tr_tiles.append(trbf)

        # exp(start) [128, 2] f32; exp(stop) [128, 2] bf16
        ssraw = stage.tile([128, 2 * JCN], F32, name="ssraw", tag="eraw")
        nc.sync.dma_start(
            out=ssraw[:, 0:JCN],
            in_=dram_ap(aux_t, AUX_START, [[1, 128], [128, JCN]]),
        )
        nc.sync.dma_start(
            out=ssraw[:, JCN : 2 * JCN],
            in_=dram_ap(aux_t, AUX_STOP, [[1, 128], [128, JCN]]),
        )
        sstart = const.tile([128, JCN], F32, name="sstart")
        nc.scalar.activation(
            out=sstart, in_=ssraw[:, 0:JCN], func=mybir.ActivationFunctionType.Exp
        )
        sstop = const.tile([128, JCN], BF16, name="sstop")
        nc.scalar.activation(
            out=sstop, in_=ssraw[:, JCN : 2 * JCN], func=mybir.ActivationFunctionType.Exp
        )
        ssbf = const.tile([128, 2 * JCN], BF16, name="ssbf")
        nc.vector.tensor_copy(out=ssbf, in_=ssraw)

        ones_w = const.tile([128, 128], BF16, name="ones_w")
        nc.vector.memset(ones_w, 1.0)
        ones_col = const.tile([128, 1], BF16, name="ones_col")
        nc.vector.memset(ones_col, 1.0)

        logsbuf = const.tile([BC, nspad], F32, name="logsbuf")
        nc.vector.memset(logsbuf, 1.0)  # log(1)=0 padding

        # ---------------- numerator: one-hot masks ----------------
        # OH_jc[p, t*BC + b] = 1.0 iff tags[b, t] == jc*128 + p, bf16,
        # with BC zero columns of padding at t == length (for the t+1 shift).
        iota_sb = const.tile([128, 1], F32, name="iota_sb")
        nc.sync.dma_start(out=iota_sb, in_=iota_t[:])
        tags_bc = stage.tile([128, length * BC], F32, name="tags_bc", tag="tags_bc")
        nc.sync.dma_start(
            out=tags_bc,
            in_=dram_ap(tags_t, 0, [[0, 128], [1, length * BC]]),
        )
        noh = (length + 1) * BC
        oh_tiles = []
        for jc in range(JCN):
            oh = const.tile([128, noh], BF16, name=f"oh{jc}")
            oh_engine = nc.gpsimd if use_gpsimd_oh else nc.vector
            oh_engine.tensor_scalar(
                out=oh[:, 0 : length * BC],
                in0=tags_bc,
                scalar1=float(jc * 128),
                scalar2=iota_sb[:],
                op0=mybir.AluOpType.subtract,
                op1=mybir.AluOpType.is_equal,
            )
            nc.vector.memset(oh[:, length * BC : noh], 0.0)
            oh_tiles.append(oh)

        # per-(tag-partition, b) accumulators for emission+transition scores:
        # each fused multiply-accumulate call writes its partial sum into a
        # distinct column; folded at the end with two strided reduces.
        tch = min(TCH, length)
        tblk = min(512, length)
        ne_calls = (length // tch) * JCN
        nt_calls = (length // tblk) * JCN
        acc2e = const.tile([128, ne_calls * BC], F32, name="acc2e")
        acc2t = const.tile([128, nt_calls * BC], F32, name="acc2t")
        scr = const.tile([128, 512], BF16, name="scr")

        # ---------------- emissions -> g = exp(emissions), bf16 ----------------
        gbuf = gpool.tile([128, length, JCN, BC], BF16, name="gbuf")
        for tci in range(length // tch):
            raw = stage.tile([128, tch, JCN, BC], F32, name="raw", tag="raw")
            # em is host-pretransposed to [p, t, jc, b]: fully contiguous load
            row = length * JCN * BC
            nc.sync.dma_start(
                out=raw,
                in_=dram_ap(
                    em_t,
                    tci * tch * JCN * BC,
                    [[row, 128], [1, tch * JCN * BC]],
                ),
            )
            nc.scalar.activation(
                out=gbuf[:, tci * tch : (tci + 1) * tch, :, :],
                in_=raw,
                func=mybir.ActivationFunctionType.Exp,
            )
            # emission part of the gold score: sum_t raw[p,(t,jc,b)]*OH[p,t,b]
            for jc in range(JCN if do_emis else 0):
                for b in range(BC):
                    c0 = tci * tch * BC + b
                    acol = (tci * JCN + jc) * BC + b
                    nc.vector.scalar_tensor_tensor(
                        out=scr[:, 0:tch],
                        in0=raw[:, :, jc, b],
                        scalar=1.0,
                        in1=oh_tiles[jc][:, c0 : c0 + (tch - 1) * BC + 1 : BC],
                        op0=mybir.AluOpType.mult,
                        op1=mybir.AluOpType.mult,
                        accum_out=acc2e[:, acol : acol + 1],
                    )

        # ---------------- the scan: NCH independent batch-chains ----------------
        # Each chain owns BCH sequences; chains interleave on the PE so the
        # per-step latency chain (mm drain -> VE multiply -> next mm) of one
        # chain hides inside the other's.
        NCH = 2
        BCH = BC // NCH
        bsl = [slice(ch * BCH, (ch + 1) * BCH) for ch in range(NCH)]

        fs = []
        for ch in range(NCH):
            f = fpool.tile([128, JCN, BCH], BF16, name=f"f{ch}", tag=f"f{ch}")
            for jc in range(JCN):
                nc.vector.tensor_scalar_mul(
                    out=f[:, jc, :],
                    in0=gbuf[:, 0, jc, bsl[ch]],
                    scalar1=sstart[:, jc : jc + 1],
                )
            fs.append(f)

        logsbufs = []
        for ch in range(NCH):
            lsb = const.tile([BCH, nspad], F32, name=f"logsbuf{ch}")
            nc.vector.memset(lsb, 1.0)
            logsbufs.append(lsb)

        msamp = 0
        for t in range(1, length):
            sample = t % RS == 0 and t + DEFER <= length - 1
            for ch in range(NCH):
                f = fs[ch]
                p = ppool.tile([128, JCN, BCH], F32, name=f"p{ch}", tag=f"p{ch}")
                for jc in range(JCN):
                    nc.tensor.matmul(
                        out=p[:, jc, :],
                        lhsT=e_tiles[0][:, jc * 128 : (jc + 1) * 128],
                        rhs=f[:, 0, :],
                        start=True,
                        stop=False,
                    )
                    nc.tensor.matmul(
                        out=p[:, jc, :],
                        lhsT=e_tiles[1][:, jc * 128 : (jc + 1) * 128],
                        rhs=f[:, 1, :],
                        start=False,
                        stop=True,
                    )
                fn = fpool.tile([128, JCN, BCH], BF16, name=f"f{ch}", tag=f"f{ch}")
                nc.vector.tensor_tensor(
                    out=fn[:],
                    in0=p[:],
                    in1=gbuf[:, t, :, bsl[ch]],
                    op=mybir.AluOpType.mult,
                )
                fs[ch] = fn

                if sample:
                    f = fn
                    s_bc = spool.tile([128, BCH], F32, name="s_bc", tag="s")
                    nc.tensor.matmul(
                        out=s_bc, lhsT=ones_w, rhs=f[:, 0, :], start=True, stop=False
                    )
                    nc.tensor.matmul(
                        out=s_bc, lhsT=ones_w, rhs=f[:, 1, :], start=False, stop=True
                    )
                    s4 = smallp.tile([BCH, 1], F32, name="s4", tag="small")
                    nc.tensor.matmul(
                        out=s4, lhsT=f[:, 0, :], rhs=ones_col, start=True, stop=False
                    )
                    nc.tensor.matmul(
                        out=s4, lhsT=f[:, 1, :], rhs=ones_col, start=False, stop=True
                    )
                    v = vpool.tile([128, BCH], F32, name="v", tag="v")
                    nc.vector.reciprocal(out=v, in_=s_bc)
                    nc.vector.tensor_copy(
                        out=logsbufs[ch][:, msamp : msamp + 1], in_=s4
                    )
                    for jc in range(JCN):
                        nc.vector.tensor_mul(
                            out=gbuf[:, t + DEFER, jc, bsl[ch]],
                            in0=gbuf[:, t + DEFER, jc, bsl[ch]],
                            in1=v,
                        )
            if sample:
                msamp += 1
        assert msamp == nsamp

        # ---------------- transition part of the gold score ----------------
        # y[j',(t,b)] = sum_i Tr[i,j'] * OH_t[i,(t,b)]; then
        # sum_t y[j',(t,b)] * OH_{t+1}[j',(t,b)] accumulated into numacc.
        for b in range(BC if do_trans else 0):
            for tc2 in range(length // tblk):
                for jcp in range(JCN):
                    y_ps = ppool.tile([128, tblk], F32, name="y_ps", tag="p0")
                    c0 = tc2 * tblk * BC + b
                    for ic in range(JCN):
                        nc.tensor.matmul(
                            out=y_ps,
                            lhsT=tr_tiles[ic][:, jcp * 128 : (jcp + 1) * 128],
                            rhs=oh_tiles[ic][:, c0 : c0 + (tblk - 1) * BC + 1 : BC],
                            start=(ic == 0),
                            stop=(ic == JCN - 1),
                        )
                    c1 = c0 + BC  # t+1 shift (zero-padded past t=length-1)
                    acol = (tc2 * JCN + jcp) * BC + b
                    nc.vector.scalar_tensor_tensor(
                        out=scr[:, 0:tblk],
                        in0=y_ps,
                        scalar=1.0,
                        in1=oh_tiles[jcp][:, c1 : c1 + (tblk - 1) * BC + 1 : BC],
                        op0=mybir.AluOpType.mult,
                        op1=mybir.AluOpType.mult,
                        accum_out=acc2t[:, acol : acol + 1],
                    )

        # fold the per-call partial sums into numacc [128, BC]
        numacc = const.tile([128, BC], F32, name="numacc")
        rede = const.tile([128, BC], F32, name="rede")
        e_view = bass.AP(
            tensor=acc2e.tensor,
            offset=acc2e.offset,
            ap=[acc2e.ap[0], [1, BC], [BC, ne_calls]],
        )
        nc.vector.tensor_reduce(
            out=rede, in_=e_view, axis=mybir.AxisListType.X, op=mybir.AluOpType.add
        )
        t_view = bass.AP(
            tensor=acc2t.tensor,
            offset=acc2t.offset,
            ap=[acc2t.ap[0], [1, BC], [BC, nt_calls]],
        )
        redt = const.tile([128, BC], F32, name="redt")
        nc.vector.tensor_reduce(
            out=redt, in_=t_view, axis=mybir.AxisListType.X, op=mybir.AluOpType.add
        )
        nc.vector.tensor_add(out=numacc, in0=rede, in1=redt)

        ones_col_f = const.tile([128, 1], F32, name="ones_col_f")
        nc.vector.memset(ones_col_f, 1.0)

        # ---------------- finalization (per chain) ----------------
        for ch in range(NCH):
            f = fs[ch]
            fin = smallp.tile([BCH, 1], F32, name=f"fin{ch}", tag="small")
            nc.tensor.matmul(
                out=fin, lhsT=f[:, 0, :], rhs=sstop[:, 0:1], start=True, stop=False
            )
            nc.tensor.matmul(
                out=fin, lhsT=f[:, 1, :], rhs=sstop[:, 1:2], start=False, stop=True
            )
            # numerator for this chain's sequences
            numer_ps = smallp.tile([BCH, 1], F32, name=f"numer_ps{ch}", tag="small")
            nc.tensor.matmul(
                out=numer_ps,
                lhsT=numacc[:, bsl[ch]],
                rhs=ones_col_f,
                start=True,
                stop=not do_ssmm,
            )
            lastc = (length - 1) * BC
            for jc in range(JCN if do_ssmm else 0):
                nc.tensor.matmul(
                    out=numer_ps,
                    lhsT=oh_tiles[jc][:, ch * BCH : ch * BCH + BCH],
                    rhs=ssbf[:, jc : jc + 1],
                    start=False,
                    stop=False,
                )
                nc.tensor.matmul(
                    out=numer_ps,
                    lhsT=oh_tiles[jc][:, lastc + ch * BCH : lastc + ch * BCH + BCH],
                    rhs=ssbf[:, JCN + jc : JCN + jc + 1],
                    start=False,
                    stop=(jc == JCN - 1),
                )

            logtmp = const.tile([BCH, nspad], F32, name=f"logtmp{ch}")
            sumlog = const.tile([BCH, 1], F32, name=f"sumlog{ch}")
            nc.scalar.activation(
                out=logtmp,
                in_=logsbufs[ch],
                func=mybir.ActivationFunctionType.Ln,
                accum_out=sumlog,
            )
            logfin = const.tile([BCH, 1], F32, name=f"logfin{ch}")
            nc.scalar.activation(
                out=logfin, in_=fin, func=mybir.ActivationFunctionType.Ln
            )
            t3 = const.tile([BCH, 1], F32, name=f"t3{ch}")
            nc.vector.tensor_sub(out=t3, in0=numer_ps, in1=logfin)
            loss_sb = const.tile([BCH, 1], F32, name=f"loss_sb{ch}")
            # numerator used unshifted transitions; the E-side folded -CSH per
            # step: loss = t3 - CSH*(L-1) - sumlog
            nc.vector.scalar_tensor_tensor(
                out=loss_sb,
                in0=t3,
                scalar=float(CSH * (length - 1)),
                in1=sumlog,
                op0=mybir.AluOpType.subtract,
                op1=mybir.AluOpType.subtract,
            )
            nc.sync.dma_start(
                out=dram_ap(loss_t, ch * BCH, [[1, BCH], [1, 1]]), in_=loss_sb
            )

    nc.finalize()
    return nc


def host_inputs(inputs, tags, length=L):
    """Build per-core input maps (host-side sharding / layout prep only)."""
    inputs = np.asarray(inputs, dtype=np.float32)
    tags = np.asarray(tags)

    in_maps = []
    for c in range(NCORES):
        bsl = slice(c * BC, (c + 1) * BC)
        # pretranspose (layout only) to [j%128, t, j//128, b] so device loads
        # are fully contiguous per partition
        em = np.ascontiguousarray(
            inputs[bsl].reshape(BC, length, JCN, 128).transpose(3, 1, 2, 0)
        ).reshape(BC * length * T, 1)
        # tags in (t, b) order as f32 (exact for tag ids < 2^24)
        tg = np.ascontiguousarray(tags[bsl].T).astype(np.float32)
        in_maps.append(dict(em=em, tags_tb=tg.reshape(length * BC, 1)))
    return in_maps


def host_shared(transitions, start_transitions, stop_transitions):
    aux = np.zeros((AUX_N, 1), dtype=np.float32)
    # shifted by -CSH: cancels between numerator gathers and log-partition
    aux[: T * T, 0] = np.asarray(transitions, dtype=np.float32).reshape(-1) - CSH
    aux[AUX_START : AUX_START + T, 0] = np.asarray(start_transitions, np.float32)
    aux[AUX_STOP : AUX_STOP + T, 0] = np.asarray(stop_transitions, np.float32)
    aux[AUX_TRRAW :, 0] = np.asarray(transitions, dtype=np.float32).reshape(-1)
    iota = np.arange(128, dtype=np.float32).reshape(128, 1)
    return dict(aux=aux, iota=iota)


def kernel(inputs, tags, mask, transitions, start_transitions, stop_transitions):
    del mask  # all-ones per the problem spec
    in_maps = host_inputs(inputs, tags)
    shared = host_shared(transitions, start_transitions, stop_transitions)
    for m in in_maps:
        m.update(shared)

    nc = build_program()
    res = run_bass_kernel_spmd(nc, in_maps, core_ids=list(range(NCORES)))
    out = np.concatenate([r["loss"].reshape(BC) for r in res.results])
    return out.astype(np.float32)


if __name__ == "__main__":
    rng = np.random.default_rng(0)
    inputs = rng.standard_normal((B, L, T), dtype=np.float32)
    tags = rng.integers(0, T, size=(B, L))
    trans = rng.standard_normal((T, T)).astype(np.float32)
    start = rng.standard_normal(T).astype(np.float32)
    stop = rng.standard_normal(T).astype(np.float32)
    out = kernel(inputs, tags, np.ones((B, L), bool), trans, start, stop)
    print(out)



# revision 21
# speedup vs baseline: 1.8433x; 1.8433x over previous
"""ConditionalRandomField loss kernel for Trainium2 (8 NeuronCores).

Math (per sequence b):
    loss[b] = log_score(gold path) - log_partition

log_partition via a meet-in-the-middle linear scan in exp space:
    fwd:  F_t = (E^T F_{t-1}) * g_t        t = 1..511,  F_0 = exp(start)*g_0
    bwd:  B_t = E (g_t * B_{t+1})          t = 1023..512, B_1024 = exp(stop)
    Z    = sum_j F_511[j] * B_512[j]
with E = exp(transitions) in fp8e4m3 (PE weights) and g_t = exp(emit_t - S)
(S = 6.5 folded shift keeps the running product in bf16 range with no
per-step rescaling; log Z = ln(Z_hat) + 1024*S).  Halves the sequential
depth to 512 steps, and the fwd/bwd chains hide each other's
PE->PSUM->DVE->PE round-trip latency.

Emissions/tags are host-relaid in "slot" order: slot k columns 0-7 hold
t=k (fwd), columns 8-15 hold t=1024-k (bwd), so one sequential DMA feeds
both chains from slot 0 upward and the numerator indexing stays uniform.

The gold-path numerator uses one-hot tag masks (built on device from an
iota compare): emissions via fused multiply-accumulate against the raw
emission chunks, transitions[tag_t, tag_t+1] via y = Tr^T @ OH matmuls
followed by a masked accumulate against the +-1-slot-shifted one-hot,
start/stop via tiny matmuls.

Sharding: data-parallel over batch; core c owns sequences [8c, 8c+8).

NOTE: mask is all-ones for this problem spec (fill: ones); the kernel
assumes it (the reference's masked branches are identities then).
"""

import numpy as np
from contextlib import ExitStack

import concourse.bass as bass
import concourse.bacc as bacc
import concourse.tile as tile
from concourse import mybir
from concourse.bass_utils import run_bass_kernel_spmd

F32 = mybir.dt.float32
BF16 = mybir.dt.bfloat16
FP8 = mybir.dt.float8e4

NCORES = 8
B = 64
L = 1024
T = 256
BC = B // NCORES      # sequences per core
JCN = T // 128        # = 2 tag chunks
NK = L // 2           # scan iterations (fwd+bwd per iteration)
SLOTS = NK + 1        # emission slots (slot k: fwd t=k | bwd t=1024-k)
NCOL = 2 * BC         # 16 columns per slot (fwd 8 | bwd 8)
S = 6.5               # log-shift folded into g = exp(emit - S)
DUMMY_TAG = 999.0     # never matches a one-hot row

AUX_TT = T * T        # aux: [trans i-major | trans j-major | start | stop]
AUX_SS = 2 * T * T
AUX_N = 2 * T * T + 2 * T

CHUNK_BOUNDS = [0, 129, 257, 385, 513]   # slot chunks for the em load


class _Bacc(bacc.Bacc):
    def __init__(self, move_waits=True):
        super().__init__()
        self._move_waits = move_waits

    def move_matmul_waits_to_ldweights(self):
        # Moving extra MM waits onto LDWEIGHTS blocks weight prefetch during
        # the DVE phase; disabled, the framework splits waits via
        # EVENT_SEMAPHORE and the (data-independent) LDW can run early.
        if self._move_waits:
            super().move_matmul_waits_to_ldweights()


def build_program(move_waits=True, debug=False):
    nc = _Bacc(move_waits=move_waits)
    nch_ = len(CHUNK_BOUNDS) - 1
    em_t = nc.declare_dram_parameter(
        "em", [128 * nch_ * 2 * NCOL * CHUNK_BOUNDS[1], 1], F32, isOutput=False
    )
    aux_t = nc.declare_dram_parameter("aux", [AUX_N, 1], F32, isOutput=False)
    tags_t = nc.declare_dram_parameter("tags_sc", [SLOTS * NCOL, 1], F32, isOutput=False)
    iota_t = nc.declare_dram_parameter("iota", [128, 1], F32, isOutput=False)
    loss_t = nc.declare_dram_parameter("loss", [BC, 1], F32, isOutput=True)
    dbg_t = nc.declare_dram_parameter("dbg", [128 * 128, 1], F32, isOutput=True) if debug else None

    def dram_ap(handle, offset, ap):
        full = handle[:]
        return bass.AP(tensor=full.tensor, offset=offset, ap=ap)

    with tile.TileContext(nc) as tc, ExitStack() as ctx:
        const = ctx.enter_context(tc.tile_pool(name="const", bufs=1))
        stage = ctx.enter_context(tc.tile_pool(name="stage", bufs=3))
        tpool = ctx.enter_context(tc.tile_pool(name="tpool", bufs=1))
        gpool = ctx.enter_context(tc.tile_pool(name="gpool", bufs=1))
        fpool = ctx.enter_context(tc.tile_pool(name="fpool", bufs=3))
        upool = ctx.enter_context(tc.tile_pool(name="upool", bufs=3))
        pfpool = ctx.enter_context(tc.tile_pool(name="pfpool", bufs=2, space="PSUM"))
        pbpool = ctx.enter_context(tc.tile_pool(name="pbpool", bufs=2, space="PSUM"))
        ypool = ctx.enter_context(tc.tile_pool(name="ypool", bufs=2, space="PSUM"))
        smallp = ctx.enter_context(tc.tile_pool(name="smallp", bufs=2, space="PSUM"))

        # ---------------- constants ----------------
        iota_sb = const.tile([128, 1], F32, name="iota_sb")
        nc.sync.dma_start(out=iota_sb, in_=iota_t[:])

        # E tiles: exp(trans) fp8, i-chunk major; TR tiles: raw trans bf16.
        e_tiles, tr_tiles = [], []
        for ic in range(JCN):
            eraw = stage.tile([128, T], F32, name=f"eraw{ic}", tag="eraw")
            nc.sync.dma_start(
                out=eraw, in_=dram_ap(aux_t, ic * 128 * T, [[T, 128], [1, T]])
            )
            ebf = const.tile([128, T], FP8, name=f"ebf{ic}")
            nc.scalar.activation(out=ebf, in_=eraw, func=mybir.ActivationFunctionType.Exp)
            e_tiles.append(ebf)
            trbf = const.tile([128, T], BF16, name=f"trbf{ic}")
            nc.vector.tensor_copy(out=trbf, in_=eraw)
            tr_tiles.append(trbf)
        # ET tiles: exp(trans)^T fp8, j-chunk major (for the bwd chain).
        et_tiles = []
        for jc in range(JCN):
            eraw = stage.tile([128, T], F32, name=f"etraw{jc}", tag="eraw")
            nc.sync.dma_start(
                out=eraw,
                in_=dram_ap(aux_t, AUX_TT + jc * 128 * T, [[T, 128], [1, T]]),
            )
            etbf = const.tile([128, T], FP8, name=f"etbf{jc}")
            nc.scalar.activation(out=etbf, in_=eraw, func=mybir.ActivationFunctionType.Exp)
            et_tiles.append(etbf)

        # start/stop: raw bf16 (numerator) + exp f32 (scan boundary values)
        ssraw = stage.tile([128, 2 * JCN], F32, name="ssraw", tag="eraw")
        nc.sync.dma_start(
            out=ssraw[:, 0:JCN], in_=dram_ap(aux_t, AUX_SS, [[1, 128], [128, JCN]])
        )
        nc.sync.dma_start(
            out=ssraw[:, JCN : 2 * JCN],
            in_=dram_ap(aux_t, AUX_SS + T, [[1, 128], [128, JCN]]),
        )
        ssbf = const.tile([128, 2 * JCN], BF16, name="ssbf")
        nc.vector.tensor_copy(out=ssbf, in_=ssraw)
        sstart = const.tile([128, JCN], F32, name="sstart")
        nc.scalar.activation(
            out=sstart, in_=ssraw[:, 0:JCN], func=mybir.ActivationFunctionType.Exp
        )
        sstop = const.tile([128, JCN], F32, name="sstop")
        nc.scalar.activation(
            out=sstop, in_=ssraw[:, JCN : 2 * JCN], func=mybir.ActivationFunctionType.Exp
        )
        ones8 = const.tile([128, BC], BF16, name="ones8")
        nc.vector.memset(ones8, 1.0)
        ones_col = const.tile([128, 1], BF16, name="ones_col")
        nc.vector.memset(ones_col, 1.0)
        ones_col_f = const.tile([128, 1], F32, name="ones_col_f")
        nc.vector.memset(ones_col_f, 1.0)
        neg_shift = const.tile([128, 1], F32, name="neg_shift")
        nc.vector.memset(neg_shift, -S)
        # B_1024 = exp(stop) replicated over the 8 bwd columns
        bstop = const.tile([128, JCN, BC], BF16, name="bstop")
        for jc in range(JCN):
            nc.vector.tensor_scalar_mul(
                out=bstop[:, jc, :], in0=ones8, scalar1=sstop[:, jc : jc + 1]
            )

        # ---------------- one-hot masks (built chunked, inside the scan) --
        # OH_jc[p, s*16+c] = 1.0 iff tags_sc[s, c] == jc*128 + p
        tags_bc = tpool.tile([128, SLOTS * NCOL], F32, name="tags_bc")
        nc.sync.dma_start(
            out=tags_bc, in_=dram_ap(tags_t, 0, [[0, 128], [1, SLOTS * NCOL]])
        )
        oh_tiles = [
            const.tile([128, SLOTS * NCOL], BF16, name=f"oh{jc}") for jc in range(JCN)
        ]

        def build_oh_chunk(ci):
            s0, s1 = CHUNK_BOUNDS[ci], CHUNK_BOUNDS[ci + 1]
            for jc in range(JCN):
                nc.vector.tensor_scalar(
                    out=oh_tiles[jc][:, s0 * NCOL : s1 * NCOL],
                    in0=tags_bc[:, s0 * NCOL : s1 * NCOL],
                    scalar1=float(jc * 128),
                    scalar2=iota_sb[:],
                    op0=mybir.AluOpType.subtract,
                    op1=mybir.AluOpType.is_equal,
                )

        # ---------------- emissions: load + exp (gathers run in-scan) -----
        nch = len(CHUNK_BOUNDS) - 1
        acc2e = const.tile([128, nch * 2 * NCOL], F32, name="acc2e")
        acc2t = const.tile([128, 2 * NCOL], F32, name="acc2t")
        scr_g = const.tile([128, CHUNK_BOUNDS[1]], BF16, name="scr_g")
        scr_v = const.tile([128, NK], BF16, name="scr_v")

        # raw chunks are host-relaid [jc, c, s_local] so the gather reads are
        # contiguous; the (idle) ACT engine absorbs the strided gbuf writes.
        gbuf = gpool.tile([128, SLOTS, JCN, NCOL], BF16, name="gbuf")
        W0 = CHUNK_BOUNDS[1]
        row = nch * 2 * NCOL * W0
        raw_tiles = []
        for ci in range(nch):
            s0, s1 = CHUNK_BOUNDS[ci], CHUNK_BOUNDS[ci + 1]
            w = s1 - s0
            raw = stage.tile([128, JCN, NCOL, W0], F32, name="raw", tag="raw")
            nc.sync.dma_start(
                out=raw,
                in_=dram_ap(
                    em_t, ci * 2 * NCOL * W0, [[row, 128], [1, 2 * NCOL * W0]]
                ),
            )
            gb_out = bass.AP(
                tensor=gbuf.tensor,
                offset=gbuf.offset + s0 * 2 * NCOL,
                ap=[gbuf.ap[0], [NCOL, JCN], [1, NCOL], [2 * NCOL, w]],
            )
            raw_in = bass.AP(
                tensor=raw.tensor,
                offset=raw.offset,
                ap=[raw.ap[0], [W0 * NCOL, JCN], [W0, NCOL], [1, w]],
            )
            nc.scalar.activation(
                out=gb_out,
                in_=raw_in,
                func=mybir.ActivationFunctionType.Exp,
                bias=neg_shift[:],
            )
            raw_tiles.append(raw)

        def emit_gather(ci, jc, c):
            # gold emission: acc += sum_s raw[p, jc, c, s] * OH[p, s*16+c]
            # fwd cols use slots 0..511, bwd cols slots 1..512 (exact cover).
            s0, s1 = CHUNK_BOUNDS[ci], CHUNK_BOUNDS[ci + 1]
            a = max(s0, 1) if c >= BC else s0
            b_ = s1 if c >= BC else min(s1, NK)
            n = b_ - a
            if n <= 0:
                return
            acol = (ci * 2 + jc) * NCOL + c
            nc.vector.scalar_tensor_tensor(
                out=scr_g[:, 0:n],
                in0=raw_tiles[ci][:, jc, c, a - s0 : b_ - s0],
                scalar=1.0,
                in1=oh_tiles[jc][:, a * NCOL + c : (b_ - 1) * NCOL + c + 1 : NCOL],
                op0=mybir.AluOpType.mult,
                op1=mybir.AluOpType.mult,
                accum_out=acc2e[:, acol : acol + 1],
            )

        # DVE side-work schedule: chunk-c OH build at k_oh[c], then that
        # chunk's 32 gathers paced 1 per 3 iterations (keeps the scan's
        # mult chain from stalling behind bursts).
        k_oh = [1, 130, 258, 386]
        side_work = {}   # k -> list of thunks
        for ci in range(nch):
            side_work.setdefault(k_oh[ci], []).append(("oh", ci))
            i = 0
            for jc in range(JCN):
                for c in range(NCOL):
                    kk = k_oh[ci] + 1 + 3 * i
                    side_work.setdefault(kk, []).append(("gather", ci, jc, c))
                    i += 1

        # ---------------- the scan ----------------
        fw = fpool.tile([128, JCN, BC], BF16, name="fw", tag="fw")
        for jc in range(JCN):
            nc.vector.tensor_scalar_mul(
                out=fw[:, jc, :],
                in0=gbuf[:, 0, jc, 0:BC],
                scalar1=sstart[:, jc : jc + 1],
            )

        def dbg_dump(col, tile_in, n=NCOL):
            if dbg_t is None:
                return
            d = const.tile([128, n], F32, name=f"dbg{col}")
            nc.vector.tensor_copy(out=d, in_=tile_in)
            nc.sync.dma_start(
                out=dram_ap(dbg_t, col, [[128, 128], [1, n]]), in_=d
            )

        if debug:
            dbg_dump(0, gbuf[:, 1, :, 0:BC])
            dbg_dump(16, gbuf[:, 1, :, BC:NCOL])
            dbg_dump(32, gbuf[:, 256, :, 0:BC])
            dbg_dump(48, gbuf[:, 512, :, BC:NCOL])
            dbg_dump(64, fw)

        pb = None
        pf = None
        fw_pend = None   # fw(k-1) rhs for the pending fwd group

        def emit_side(k):
            for work in side_work.get(k, ()):
                if work[0] == "oh":
                    build_oh_chunk(work[1])
                else:
                    emit_gather(work[1], work[2], work[3])

        def emit_fwd_group(rhs):
            p = pfpool.tile([128, JCN, BC], F32, name="pf", tag="pf")
            nc.tensor.matmul(out=p[:, 0, :], lhsT=e_tiles[0][:, 0:128], rhs=rhs[:, 0, :], start=True, stop=False)
            nc.tensor.matmul(out=p[:, 0, :], lhsT=e_tiles[1][:, 0:128], rhs=rhs[:, 1, :], start=False, stop=True)
            nc.tensor.matmul(out=p[:, 1, :], lhsT=e_tiles[0][:, 128:256], rhs=rhs[:, 0, :], start=True, stop=False)
            nc.tensor.matmul(out=p[:, 1, :], lhsT=e_tiles[1][:, 128:256], rhs=rhs[:, 1, :], start=False, stop=True)
            return p

        # skewed pipeline: per iteration k emit
        #   [PE fwd_group(k-1)] [DVE mult_b(k)] [PE bwd_group(k)] [DVE mult_f(k-1)]
        # so each PE group has exactly one mult+drain ahead of it, and the
        # two DVE mults never sit back-to-back on the critical path.
        for k in range(1, NK + 1):
            emit_side(k)
            if k >= 2:
                pf = emit_fwd_group(fw)

            u = upool.tile([128, JCN, BC], BF16, name="u", tag="u")
            nc.vector.tensor_tensor(
                out=u,
                in0=(bstop if k == 1 else pb),
                in1=gbuf[:, k, :, BC:NCOL],
                op=mybir.AluOpType.mult,
            )
            if k == NK // 2:
                # one mid-scan 2^-24 rescale per chain keeps the final dot
                # product inside the ACT Ln table range (breaks above ~1e17)
                us = upool.tile([128, JCN, BC], BF16, name="u", tag="u")
                nc.vector.tensor_scalar_mul(out=us, in0=u, scalar1=2.0 ** -24)
                u = us
            if debug and k in (2, 64, 256, 400):
                dbg_dump({2: 80, 64: 84, 256: 88, 400: 92}[k], u[:, :, 0:2], 4)

            pb = pbpool.tile([128, JCN, BC], F32, name="pb", tag="pb")
            nc.tensor.matmul(out=pb[:, 0, :], lhsT=et_tiles[0][:, 0:128], rhs=u[:, 0, :], start=True, stop=False)
            nc.tensor.matmul(out=pb[:, 0, :], lhsT=et_tiles[1][:, 0:128], rhs=u[:, 1, :], start=False, stop=True)
            nc.tensor.matmul(out=pb[:, 1, :], lhsT=et_tiles[0][:, 128:256], rhs=u[:, 0, :], start=True, stop=False)
            nc.tensor.matmul(out=pb[:, 1, :], lhsT=et_tiles[1][:, 128:256], rhs=u[:, 1, :], start=False, stop=True)

            if k >= 2:
                j = k - 1
                fw2 = fpool.tile([128, JCN, BC], BF16, name="fw", tag="fw")
                nc.vector.tensor_tensor(
                    out=fw2, in0=pf, in1=gbuf[:, j, :, 0:BC], op=mybir.AluOpType.mult
                )
                fw = fw2
                if debug and j in (2, 64, 256, 400):
                    dbg_dump({2: 96, 64: 100, 256: 104, 400: 108}[j], fw[:, :, 0:2], 4)
                if j == NK // 2:
                    fws = fpool.tile([128, JCN, BC], BF16, name="fw", tag="fw")
                    nc.vector.tensor_scalar_mul(out=fws, in0=fw, scalar1=2.0 ** -24)
                    fw = fws

        # ---------------- gold transition scores ----------------
        # y[j', s] = Tr[tag_s, j']; acc += sum_s y[j', s] * OH_{s+-1}[j', s]
        for side in range(2):
            for b in range(BC):
                c = side * BC + b
                if side == 0:
                    sa, n, shift = 0, NK, NCOL          # slots 0..511, next t at +1 slot
                else:
                    sa, n, shift = 2, NK - 1, -NCOL     # slots 2..512, next t at -1 slot
                base = sa * NCOL + c
                for jcp in range(JCN):
                    y_ps = ypool.tile([128, NK], F32, name="y_ps", tag="y")
                    for ic in range(JCN):
                        nc.tensor.matmul(
                            out=y_ps[:, 0:n],
                            lhsT=tr_tiles[ic][:, jcp * 128 : (jcp + 1) * 128],
                            rhs=oh_tiles[ic][:, base : base + (n - 1) * NCOL + 1 : NCOL],
                            start=(ic == 0),
                            stop=(ic == JCN - 1),
                        )
                    acol = jcp * NCOL + c
                    nc.vector.scalar_tensor_tensor(
                        out=scr_v[:, 0:n],
                        in0=y_ps[:, 0:n],
                        scalar=1.0,
                        in1=oh_tiles[jcp][:, base + shift : base + shift + (n - 1) * NCOL + 1 : NCOL],
                        op0=mybir.AluOpType.mult,
                        op1=mybir.AluOpType.mult,
                        accum_out=acc2t[:, acol : acol + 1],
                    )

        # ---------------- finalization ----------------
        # Z = sum_j F_511 * B_512 (per column pair)
        h = const.tile([128, JCN, BC], BF16, name="h")
        nc.vector.tensor_tensor(out=h, in0=pb, in1=fw, op=mybir.AluOpType.mult)
        if debug:
            dbg_dump(112, h)
        s4 = smallp.tile([BC, 1], F32, name="s4", tag="small")
        nc.tensor.matmul(out=s4, lhsT=h[:, 0, :], rhs=ones_col, start=True, stop=False)
        nc.tensor.matmul(out=s4, lhsT=h[:, 1, :], rhs=ones_col, start=False, stop=True)
        logfin = const.tile([BC, 1], F32, name="logfin")
        nc.scalar.activation(out=logfin, in_=s4, func=mybir.ActivationFunctionType.Ln)

        # fold per-call accumulators: rede/redt [128, 16] col c
        rede = const.tile([128, NCOL], F32, name="rede")
        e_view = bass.AP(
            tensor=acc2e.tensor,
            offset=acc2e.offset,
            ap=[acc2e.ap[0], [1, NCOL], [NCOL, nch * 2]],
        )
        nc.vector.tensor_reduce(
            out=rede, in_=e_view, axis=mybir.AxisListType.X, op=mybir.AluOpType.add
        )
        redt = const.tile([128, NCOL], F32, name="redt")
        t_view = bass.AP(
            tensor=acc2t.tensor,
            offset=acc2t.offset,
            ap=[acc2t.ap[0], [1, NCOL], [NCOL, 2]],
        )
        nc.vector.tensor_reduce(
            out=redt, in_=t_view, axis=mybir.AxisListType.X, op=mybir.AluOpType.add
        )
        numacc = const.tile([128, NCOL], F32, name="numacc")
        nc.vector.tensor_add(out=numacc, in0=rede, in1=redt)

        # numer[b] = sum_p numacc[p, b] + numacc[p, 8+b]
        #          + start[tag_0] + stop[tag_1023]
        numer_ps = smallp.tile([BC, 1], F32, name="numer_ps", tag="small")
        nc.tensor.matmul(out=numer_ps, lhsT=numacc[:, 0:BC], rhs=ones_col_f, start=True, stop=False)
        nc.tensor.matmul(out=numer_ps, lhsT=numacc[:, BC:NCOL], rhs=ones_col_f, start=False, stop=False)
        nc.tensor.matmul(out=numer_ps, lhsT=oh_tiles[0][:, 0:BC], rhs=ssbf[:, 0:1], start=False, stop=False)
        nc.tensor.matmul(out=numer_ps, lhsT=oh_tiles[1][:, 0:BC], rhs=ssbf[:, 1:2], start=False, stop=False)
        nc.tensor.matmul(out=numer_ps, lhsT=oh_tiles[0][:, NCOL + BC : 2 * NCOL], rhs=ssbf[:, 2:3], start=False, stop=False)
        nc.tensor.matmul(out=numer_ps, lhsT=oh_tiles[1][:, NCOL + BC : 2 * NCOL], rhs=ssbf[:, 3:4], start=False, stop=True)

        # loss = (numer - L*S - 48*ln2) - ln(Z_hat)   (2^-48 from the rescales)
        loss_sb = const.tile([BC, 1], F32, name="loss_sb")
        nc.vector.scalar_tensor_tensor(
            out=loss_sb,
            in0=numer_ps,
            scalar=float(L * S + 48.0 * np.log(2.0)),
            in1=logfin,
            op0=mybir.AluOpType.subtract,
            op1=mybir.AluOpType.subtract,
        )
        nc.sync.dma_start(out=dram_ap(loss_t, 0, [[1, BC], [1, 1]]), in_=loss_sb)

    nc.finalize()
    return nc


def host_inputs(inputs, tags, length=L):
    """Per-core slot-relaid inputs (host-side sharding / layout prep only)."""
    inputs = np.asarray(inputs, dtype=np.float32)
    tags = np.asarray(tags)

    nch = len(CHUNK_BOUNDS) - 1
    W0 = CHUNK_BOUNDS[1]
    in_maps = []
    for cc in range(NCORES):
        bsl = slice(cc * BC, (cc + 1) * BC)
        xr = inputs[bsl].reshape(BC, length, JCN, 128)   # (8, 1024, 2, 128)
        em = np.zeros((128, SLOTS, JCN, NCOL), dtype=np.float32)
        em[:, 0:NK, :, 0:BC] = xr[:, 0:NK].transpose(3, 1, 2, 0)
        em[:, 1:NK + 1, :, BC:NCOL] = xr[:, length - 1 : NK - 1 : -1].transpose(3, 1, 2, 0)
        # chunked [jc, c, s_local] layout, each chunk padded to W0 slots
        em_ch = np.zeros((128, nch, JCN, NCOL, W0), dtype=np.float32)
        for ci in range(nch):
            s0, s1 = CHUNK_BOUNDS[ci], CHUNK_BOUNDS[ci + 1]
            em_ch[:, ci, :, :, 0 : s1 - s0] = em[:, s0:s1].transpose(0, 2, 3, 1)
        tg = np.full((SLOTS, NCOL), DUMMY_TAG, dtype=np.float32)
        tg[0:SLOTS, 0:BC] = tags[bsl][:, 0:SLOTS].T
        tg[1:NK + 1, BC:NCOL] = tags[bsl][:, length - 1 : NK - 1 : -1].T
        in_maps.append(
            dict(em=em_ch.reshape(-1, 1), tags_sc=tg.reshape(-1, 1))
        )
    return in_maps


def host_shared(transitions, start_transitions, stop_transitions):
    tr = np.asarray(transitions, dtype=np.float32)
    aux = np.zeros((AUX_N, 1), dtype=np.float32)
    aux[: T * T, 0] = tr.reshape(-1)               # i-major (fwd E tiles)
    aux[T * T : 2 * T * T, 0] = tr.T.reshape(-1)   # j-major (bwd ET tiles)
    aux[AUX_SS : AUX_SS + T, 0] = np.asarray(start_transitions, np.float32)
    aux[AUX_SS + T :, 0] = np.asarray(stop_transitions, np.float32)
    iota = np.arange(128, dtype=np.float32).reshape(128, 1)
    return dict(aux=aux, iota=iota)


def kernel(inputs, tags, mask, transitions, start_transitions, stop_transitions):
    del mask  # all-ones per the problem spec
    in_maps = host_inputs(inputs, tags)
    shared = host_shared(transitions, start_transitions, stop_transitions)
    for m in in_maps:
        m.update(shared)

    nc = build_program()
    res = run_bass_kernel_spmd(nc, in_maps, core_ids=list(range(NCORES)))
    out = np.concatenate([r["loss"].reshape(BC) for r in res.results])
    return out.astype(np.float32)


if __name__ == "__main__":
    rng = np.random.default_rng(0)
    inputs = rng.standard_normal((B, L, T), dtype=np.float32)
    tags = rng.integers(0, T, size=(B, L))
    trans = rng.standard_normal((T, T)).astype(np.float32)
    start = rng.standard_normal(T).astype(np.float32)
    stop = rng.standard_normal(T).astype(np.float32)
    out = kernel(inputs, tags, np.ones((B, L), bool), trans, start, stop)
    print(out)


# revision 23
# speedup vs baseline: 1.8464x; 1.0017x over previous
"""ConditionalRandomField loss kernel for Trainium2 (8 NeuronCores).

Math (per sequence b):
    loss[b] = log_score(gold path) - log_partition

log_partition via a meet-in-the-middle linear scan in exp space:
    fwd:  F_t = (E^T F_{t-1}) * g_t        t = 1..511,  F_0 = exp(start)*g_0
    bwd:  B_t = E (g_t * B_{t+1})          t = 1023..512, B_1024 = exp(stop)
    Z    = sum_j F_511[j] * B_512[j]
with E = exp(transitions) in fp8e4m3 (PE weights) and g_t = exp(emit_t - S)
(S = 6.5 folded shift keeps the running product in bf16 range with no
per-step rescaling; log Z = ln(Z_hat) + 1024*S).  Halves the sequential
depth to 512 steps, and the fwd/bwd chains hide each other's
PE->PSUM->DVE->PE round-trip latency.

Emissions/tags are host-relaid in "slot" order: slot k columns 0-7 hold
t=k (fwd), columns 8-15 hold t=1024-k (bwd), so one sequential DMA feeds
both chains from slot 0 upward and the numerator indexing stays uniform.

The gold-path numerator uses one-hot tag masks (built on device from an
iota compare): emissions via fused multiply-accumulate against the raw
emission chunks, transitions[tag_t, tag_t+1] via y = Tr^T @ OH matmuls
followed by a masked accumulate against the +-1-slot-shifted one-hot,
start/stop via tiny matmuls.

Sharding: data-parallel over batch; core c owns sequences [8c, 8c+8).

NOTE: mask is all-ones for this problem spec (fill: ones); the kernel
assumes it (the reference's masked branches are identities then).
"""

import numpy as np
from contextlib import ExitStack

import concourse.bass as bass
import concourse.bacc as bacc
import concourse.tile as tile
from concourse import mybir
from concourse.bass_utils import run_bass_kernel_spmd

F32 = mybir.dt.float32
BF16 = mybir.dt.bfloat16
FP8 = mybir.dt.float8e4

NCORES = 8
B = 64
L = 1024
T = 256
BC = B // NCORES      # sequences per core
JCN = T // 128        # = 2 tag chunks
NK = L // 2           # scan iterations (fwd+bwd per iteration)
SLOTS = NK + 1        # emission slots (slot k: fwd t=k | bwd t=1024-k)
NCOL = 2 * BC         # 16 columns per slot (fwd 8 | bwd 8)
S = 6.5               # log-shift folded into g = exp(emit - S)
DUMMY_TAG = 999.0     # never matches a one-hot row

AUX_TT = T * T        # aux: [trans i-major | trans j-major | start | stop]
AUX_SS = 2 * T * T
AUX_N = 2 * T * T + 2 * T

CHUNK_BOUNDS = [0, 129, 257, 385, 513]   # slot chunks for the em load


class _Bacc(bacc.Bacc):
    def __init__(self, move_waits=True):
        super().__init__()
        self._move_waits = move_waits

    def move_matmul_waits_to_ldweights(self):
        # Moving extra MM waits onto LDWEIGHTS blocks weight prefetch during
        # the DVE phase; disabled, the framework splits waits via
        # EVENT_SEMAPHORE and the (data-independent) LDW can run early.
        if self._move_waits:
            super().move_matmul_waits_to_ldweights()


def build_program(move_waits=True, debug=False):
    nc = _Bacc(move_waits=move_waits)
    nch_ = len(CHUNK_BOUNDS) - 1
    em_t = nc.declare_dram_parameter(
        "em", [128 * nch_ * 2 * NCOL * CHUNK_BOUNDS[1], 1], F32, isOutput=False
    )
    aux_t = nc.declare_dram_parameter("aux", [AUX_N, 1], F32, isOutput=False)
    tags_t = nc.declare_dram_parameter("tags_sc", [SLOTS * NCOL, 1], F32, isOutput=False)
    iota_t = nc.declare_dram_parameter("iota", [128, 1], F32, isOutput=False)
    loss_t = nc.declare_dram_parameter("loss", [BC, 1], F32, isOutput=True)
    dbg_t = nc.declare_dram_parameter("dbg", [128 * 128, 1], F32, isOutput=True) if debug else None

    def dram_ap(handle, offset, ap):
        full = handle[:]
        return bass.AP(tensor=full.tensor, offset=offset, ap=ap)

    with tile.TileContext(nc) as tc, ExitStack() as ctx:
        const = ctx.enter_context(tc.tile_pool(name="const", bufs=1))
        stage = ctx.enter_context(tc.tile_pool(name="stage", bufs=3))
        tpool = ctx.enter_context(tc.tile_pool(name="tpool", bufs=1))
        gpool = ctx.enter_context(tc.tile_pool(name="gpool", bufs=1))
        fpool = ctx.enter_context(tc.tile_pool(name="fpool", bufs=3))
        upool = ctx.enter_context(tc.tile_pool(name="upool", bufs=3))
        pfpool = ctx.enter_context(tc.tile_pool(name="pfpool", bufs=2, space="PSUM"))
        pbpool = ctx.enter_context(tc.tile_pool(name="pbpool", bufs=2, space="PSUM"))
        ypool = ctx.enter_context(tc.tile_pool(name="ypool", bufs=2, space="PSUM"))
        smallp = ctx.enter_context(tc.tile_pool(name="smallp", bufs=2, space="PSUM"))

        # ---------------- constants ----------------
        iota_sb = const.tile([128, 1], F32, name="iota_sb")
        nc.sync.dma_start(out=iota_sb, in_=iota_t[:])

        # E tiles: exp(trans) fp8, i-chunk major; TR tiles: raw trans bf16.
        e_tiles, tr_tiles = [], []
        for ic in range(JCN):
            eraw = stage.tile([128, T], F32, name=f"eraw{ic}", tag="eraw")
            nc.sync.dma_start(
                out=eraw, in_=dram_ap(aux_t, ic * 128 * T, [[T, 128], [1, T]])
            )
            ebf = const.tile([128, T], FP8, name=f"ebf{ic}")
            nc.scalar.activation(out=ebf, in_=eraw, func=mybir.ActivationFunctionType.Exp)
            e_tiles.append(ebf)
            trbf = const.tile([128, T], BF16, name=f"trbf{ic}")
            nc.vector.tensor_copy(out=trbf, in_=eraw)
            tr_tiles.append(trbf)
        # ET tiles: exp(trans)^T fp8, j-chunk major (for the bwd chain).
        et_tiles = []
        for jc in range(JCN):
            eraw = stage.tile([128, T], F32, name=f"etraw{jc}", tag="eraw")
            nc.sync.dma_start(
                out=eraw,
                in_=dram_ap(aux_t, AUX_TT + jc * 128 * T, [[T, 128], [1, T]]),
            )
            etbf = const.tile([128, T], FP8, name=f"etbf{jc}")
            nc.scalar.activation(out=etbf, in_=eraw, func=mybir.ActivationFunctionType.Exp)
            et_tiles.append(etbf)

        # start/stop: raw bf16 (numerator) + exp f32 (scan boundary values)
        ssraw = stage.tile([128, 2 * JCN], F32, name="ssraw", tag="eraw")
        nc.sync.dma_start(
            out=ssraw[:, 0:JCN], in_=dram_ap(aux_t, AUX_SS, [[1, 128], [128, JCN]])
        )
        nc.sync.dma_start(
            out=ssraw[:, JCN : 2 * JCN],
            in_=dram_ap(aux_t, AUX_SS + T, [[1, 128], [128, JCN]]),
        )
        ssbf = const.tile([128, 2 * JCN], BF16, name="ssbf")
        nc.vector.tensor_copy(out=ssbf, in_=ssraw)
        sstart = const.tile([128, JCN], F32, name="sstart")
        nc.scalar.activation(
            out=sstart, in_=ssraw[:, 0:JCN], func=mybir.ActivationFunctionType.Exp
        )
        sstop = const.tile([128, JCN], F32, name="sstop")
        nc.scalar.activation(
            out=sstop, in_=ssraw[:, JCN : 2 * JCN], func=mybir.ActivationFunctionType.Exp
        )
        ones8 = const.tile([128, BC], BF16, name="ones8")
        nc.vector.memset(ones8, 1.0)
        ones_col = const.tile([128, 1], BF16, name="ones_col")
        nc.vector.memset(ones_col, 1.0)
        ones_col_f = const.tile([128, 1], F32, name="ones_col_f")
        nc.vector.memset(ones_col_f, 1.0)
        neg_shift = const.tile([128, 1], F32, name="neg_shift")
        nc.vector.memset(neg_shift, -S)
        # B_1024 = exp(stop) replicated over the 8 bwd columns
        bstop = const.tile([128, JCN, BC], BF16, name="bstop")
        for jc in range(JCN):
            nc.vector.tensor_scalar_mul(
                out=bstop[:, jc, :], in0=ones8, scalar1=sstop[:, jc : jc + 1]
            )

        # ---------------- one-hot masks (built chunked, inside the scan) --
        # OH_jc[p, s*16+c] = 1.0 iff tags_sc[s, c] == jc*128 + p
        tags_bc = tpool.tile([128, SLOTS * NCOL], F32, name="tags_bc")
        nc.sync.dma_start(
            out=tags_bc, in_=dram_ap(tags_t, 0, [[0, 128], [1, SLOTS * NCOL]])
        )
        oh_tiles = [
            const.tile([128, SLOTS * NCOL], BF16, name=f"oh{jc}") for jc in range(JCN)
        ]

        def build_oh_piece(jc, p0, p1):
            nc.vector.tensor_scalar(
                out=oh_tiles[jc][:, p0:p1],
                in0=tags_bc[:, p0:p1],
                scalar1=float(jc * 128),
                scalar2=iota_sb[:],
                op0=mybir.AluOpType.subtract,
                op1=mybir.AluOpType.is_equal,
            )

        # ---------------- emissions: load + exp (gathers run in-scan) -----
        nch = len(CHUNK_BOUNDS) - 1
        acc2e = const.tile([128, nch * 2 * NCOL], F32, name="acc2e")
        acc2t = const.tile([128, 2 * NCOL], F32, name="acc2t")
        scr_g = const.tile([128, CHUNK_BOUNDS[1]], BF16, name="scr_g")
        scr_v = const.tile([128, NK], BF16, name="scr_v")

        # raw chunks and gbuf are both laid out [jc, c, s] so the exp and
        # the gold-emission gathers read/write contiguously; the scan's
        # per-k multiply reads 16 strided elements instead (negligible).
        gbuf = gpool.tile([128, JCN, NCOL, SLOTS], BF16, name="gbuf")
        W0 = CHUNK_BOUNDS[1]
        row = nch * 2 * NCOL * W0
        raw_tiles = []
        for ci in range(nch):
            s0, s1 = CHUNK_BOUNDS[ci], CHUNK_BOUNDS[ci + 1]
            w = s1 - s0
            raw = stage.tile([128, JCN, NCOL, W0], F32, name="raw", tag="raw")
            nc.sync.dma_start(
                out=raw,
                in_=dram_ap(
                    em_t, ci * 2 * NCOL * W0, [[row, 128], [1, 2 * NCOL * W0]]
                ),
            )
            gb_out = bass.AP(
                tensor=gbuf.tensor,
                offset=gbuf.offset + s0,
                ap=[gbuf.ap[0], [NCOL * SLOTS, JCN], [SLOTS, NCOL], [1, w]],
            )
            raw_in = bass.AP(
                tensor=raw.tensor,
                offset=raw.offset,
                ap=[raw.ap[0], [W0 * NCOL, JCN], [W0, NCOL], [1, w]],
            )
            nc.scalar.activation(
                out=gb_out,
                in_=raw_in,
                func=mybir.ActivationFunctionType.Exp,
                bias=neg_shift[:],
            )
            raw_tiles.append(raw)

        def g_slice(k, c0, c1):
            # [128, JCN, c1-c0] view of g at slot k (strided over c)
            return bass.AP(
                tensor=gbuf.tensor,
                offset=gbuf.offset + c0 * SLOTS + k,
                ap=[gbuf.ap[0], [NCOL * SLOTS, JCN], [SLOTS, c1 - c0]],
            )

        def emit_gather(ci, jc, c):
            # gold emission: acc += sum_s raw[p, jc, c, s] * OH[p, s*16+c]
            # fwd cols use slots 0..511, bwd cols slots 1..512 (exact cover).
            s0, s1 = CHUNK_BOUNDS[ci], CHUNK_BOUNDS[ci + 1]
            a = max(s0, 1) if c >= BC else s0
            b_ = s1 if c >= BC else min(s1, NK)
            n = b_ - a
            if n <= 0:
                return
            acol = (ci * 2 + jc) * NCOL + c
            nc.vector.scalar_tensor_tensor(
                out=scr_g[:, 0:n],
                in0=raw_tiles[ci][:, jc, c, a - s0 : b_ - s0],
                scalar=1.0,
                in1=oh_tiles[jc][:, a * NCOL + c : (b_ - 1) * NCOL + c + 1 : NCOL],
                op0=mybir.AluOpType.mult,
                op1=mybir.AluOpType.mult,
                accum_out=acc2e[:, acol : acol + 1],
            )

        # DVE side-work schedule, paced so no piece exceeds the per-k DVE
        # idle window: one-hot pieces (~129 cols) 1/k over k=1..~128 in chunk
        # order, then each chunk's 32 gathers 1/k once its raw tile + OH
        # chunk exist.
        side_work = {}
        kq = 1
        for ci in range(nch):
            c0, c1 = CHUNK_BOUNDS[ci] * NCOL, CHUNK_BOUNDS[ci + 1] * NCOL
            npc = 16
            step = (c1 - c0 + npc - 1) // npc
            for jc in range(JCN):
                for p0 in range(c0, c1, step):
                    side_work.setdefault(kq, []).append(
                        ("oh", jc, p0, min(p0 + step, c1))
                    )
                    kq += 1
        for ci in range(nch):
            kg = max(kq + 1, 130 + 33 * ci)
            for jc in range(JCN):
                for c in range(NCOL):
                    side_work.setdefault(kg, []).append(("gather", ci, jc, c))
                    kg += 1

        # ---------------- the scan ----------------
        fw = fpool.tile([128, JCN, BC], BF16, name="fw", tag="fw")
        for jc in range(JCN):
            nc.vector.tensor_scalar_mul(
                out=fw[:, jc, :],
                in0=bass.AP(
                    tensor=gbuf.tensor,
                    offset=gbuf.offset + jc * NCOL * SLOTS,
                    ap=[gbuf.ap[0], [SLOTS, BC]],
                ),
                scalar1=sstart[:, jc : jc + 1],
            )

        def dbg_dump(col, tile_in, n=NCOL):
            if dbg_t is None:
                return
            d = const.tile([128, n], F32, name=f"dbg{col}")
            nc.vector.tensor_copy(out=d, in_=tile_in)
            nc.sync.dma_start(
                out=dram_ap(dbg_t, col, [[128, 128], [1, n]]), in_=d
            )

        if debug:
            dbg_dump(0, g_slice(1, 0, BC))
            dbg_dump(16, g_slice(1, BC, NCOL))
            dbg_dump(32, g_slice(256, 0, BC))
            dbg_dump(48, g_slice(512, BC, NCOL))
            dbg_dump(64, fw)

        pb = None
        pf = None
        fw_pend = None   # fw(k-1) rhs for the pending fwd group

        def emit_side(k):
            for work in side_work.get(k, ()):
                if work[0] == "oh":
                    build_oh_piece(work[1], work[2], work[3])
                else:
                    emit_gather(work[1], work[2], work[3])

        def emit_fwd_group(rhs):
            p = pfpool.tile([128, JCN, BC], F32, name="pf", tag="pf")
            nc.tensor.matmul(out=p[:, 0, :], lhsT=e_tiles[0][:, 0:128], rhs=rhs[:, 0, :], start=True, stop=False)
            nc.tensor.matmul(out=p[:, 0, :], lhsT=e_tiles[1][:, 0:128], rhs=rhs[:, 1, :], start=False, stop=True)
            nc.tensor.matmul(out=p[:, 1, :], lhsT=e_tiles[0][:, 128:256], rhs=rhs[:, 0, :], start=True, stop=False)
            nc.tensor.matmul(out=p[:, 1, :], lhsT=e_tiles[1][:, 128:256], rhs=rhs[:, 1, :], start=False, stop=True)
            return p

        # skewed pipeline: per iteration k emit
        #   [PE fwd_group(k-1)] [DVE mult_b(k)] [PE bwd_group(k)] [DVE mult_f(k-1)]
        # so each PE group has exactly one mult+drain ahead of it, and the
        # two DVE mults never sit back-to-back on the critical path.
        for k in range(1, NK + 1):
            emit_side(k)
            if k >= 2:
                pf = emit_fwd_group(fw)

            u = upool.tile([128, JCN, BC], BF16, name="u", tag="u")
            nc.vector.tensor_tensor(
                out=u,
                in0=(bstop if k == 1 else pb),
                in1=g_slice(k, BC, NCOL),
                op=mybir.AluOpType.mult,
            )
            if k == NK // 2:
                # one mid-scan 2^-24 rescale per chain keeps the final dot
                # product inside the ACT Ln table range (breaks above ~1e17)
                us = upool.tile([128, JCN, BC], BF16, name="u", tag="u")
                nc.vector.tensor_scalar_mul(out=us, in0=u, scalar1=2.0 ** -24)
                u = us
            if debug and k in (2, 64, 256, 400):
                dbg_dump({2: 80, 64: 84, 256: 88, 400: 92}[k], u[:, :, 0:2], 4)

            pb = pbpool.tile([128, JCN, BC], F32, name="pb", tag="pb")
            nc.tensor.matmul(out=pb[:, 0, :], lhsT=et_tiles[0][:, 0:128], rhs=u[:, 0, :], start=True, stop=False)
            nc.tensor.matmul(out=pb[:, 0, :], lhsT=et_tiles[1][:, 0:128], rhs=u[:, 1, :], start=False, stop=True)
            nc.tensor.matmul(out=pb[:, 1, :], lhsT=et_tiles[0][:, 128:256], rhs=u[:, 0, :], start=True, stop=False)
            nc.tensor.matmul(out=pb[:, 1, :], lhsT=et_tiles[1][:, 128:256], rhs=u[:, 1, :], start=False, stop=True)

            if k >= 2:
                j = k - 1
                fw2 = fpool.tile([128, JCN, BC], BF16, name="fw", tag="fw")
                nc.vector.tensor_tensor(
                    out=fw2, in0=pf, in1=g_slice(j, 0, BC), op=mybir.AluOpType.mult
                )
                fw = fw2
                if debug and j in (2, 64, 256, 400):
                    dbg_dump({2: 96, 64: 100, 256: 104, 400: 108}[j], fw[:, :, 0:2], 4)
                if j == NK // 2:
                    fws = fpool.tile([128, JCN, BC], BF16, name="fw", tag="fw")
                    nc.vector.tensor_scalar_mul(out=fws, in0=fw, scalar1=2.0 ** -24)
                    fw = fws

        # ---------------- gold transition scores ----------------
        # y[j', s] = Tr[tag_s, j']; acc += sum_s y[j', s] * OH_{s+-1}[j', s]
        for side in range(2):
            for b in range(BC):
                c = side * BC + b
                if side == 0:
                    sa, n, shift = 0, NK, NCOL          # slots 0..511, next t at +1 slot
                else:
                    sa, n, shift = 2, NK - 1, -NCOL     # slots 2..512, next t at -1 slot
                base = sa * NCOL + c
                for jcp in range(JCN):
                    y_ps = ypool.tile([128, NK], F32, name="y_ps", tag="y")
                    for ic in range(JCN):
                        nc.tensor.matmul(
                            out=y_ps[:, 0:n],
                            lhsT=tr_tiles[ic][:, jcp * 128 : (jcp + 1) * 128],
                            rhs=oh_tiles[ic][:, base : base + (n - 1) * NCOL + 1 : NCOL],
                            start=(ic == 0),
                            stop=(ic == JCN - 1),
                        )
                    acol = jcp * NCOL + c
                    nc.vector.scalar_tensor_tensor(
                        out=scr_v[:, 0:n],
                        in0=y_ps[:, 0:n],
                        scalar=1.0,
                        in1=oh_tiles[jcp][:, base + shift : base + shift + (n - 1) * NCOL + 1 : NCOL],
                        op0=mybir.AluOpType.mult,
                        op1=mybir.AluOpType.mult,
                        accum_out=acc2t[:, acol : acol + 1],
                    )

        # ---------------- finalization ----------------
        # Z = sum_j F_511 * B_512 (per column pair)
        h = const.tile([128, JCN, BC], BF16, name="h")
        nc.vector.tensor_tensor(out=h, in0=pb, in1=fw, op=mybir.AluOpType.mult)
        if debug:
            dbg_dump(112, h)
        s4 = smallp.tile([BC, 1], F32, name="s4", tag="small")
        nc.tensor.matmul(out=s4, lhsT=h[:, 0, :], rhs=ones_col, start=True, stop=False)
        nc.tensor.matmul(out=s4, lhsT=h[:, 1, :], rhs=ones_col, start=False, stop=True)
        logfin = const.tile([BC, 1], F32, name="logfin")
        nc.scalar.activation(out=logfin, in_=s4, func=mybir.ActivationFunctionType.Ln)

        # fold per-call accumulators: rede/redt [128, 16] col c
        rede = const.tile([128, NCOL], F32, name="rede")
        e_view = bass.AP(
            tensor=acc2e.tensor,
            offset=acc2e.offset,
            ap=[acc2e.ap[0], [1, NCOL], [NCOL, nch * 2]],
        )
        nc.vector.tensor_reduce(
            out=rede, in_=e_view, axis=mybir.AxisListType.X, op=mybir.AluOpType.add
        )
        redt = const.tile([128, NCOL], F32, name="redt")
        t_view = bass.AP(
            tensor=acc2t.tensor,
            offset=acc2t.offset,
            ap=[acc2t.ap[0], [1, NCOL], [NCOL, 2]],
        )
        nc.vector.tensor_reduce(
            out=redt, in_=t_view, axis=mybir.AxisListType.X, op=mybir.AluOpType.add
        )
        numacc = const.tile([128, NCOL], F32, name="numacc")
        nc.vector.tensor_add(out=numacc, in0=rede, in1=redt)

        # numer[b] = sum_p numacc[p, b] + numacc[p, 8+b]
        #          + start[tag_0] + stop[tag_1023]
        numer_ps = smallp.tile([BC, 1], F32, name="numer_ps", tag="small")
        nc.tensor.matmul(out=numer_ps, lhsT=numacc[:, 0:BC], rhs=ones_col_f, start=True, stop=False)
        nc.tensor.matmul(out=numer_ps, lhsT=numacc[:, BC:NCOL], rhs=ones_col_f, start=False, stop=False)
        nc.tensor.matmul(out=numer_ps, lhsT=oh_tiles[0][:, 0:BC], rhs=ssbf[:, 0:1], start=False, stop=False)
        nc.tensor.matmul(out=numer_ps, lhsT=oh_tiles[1][:, 0:BC], rhs=ssbf[:, 1:2], start=False, stop=False)
        nc.tensor.matmul(out=numer_ps, lhsT=oh_tiles[0][:, NCOL + BC : 2 * NCOL], rhs=ssbf[:, 2:3], start=False, stop=False)
        nc.tensor.matmul(out=numer_ps, lhsT=oh_tiles[1][:, NCOL + BC : 2 * NCOL], rhs=ssbf[:, 3:4], start=False, stop=True)

        # loss = (numer - L*S - 48*ln2) - ln(Z_hat)   (2^-48 from the rescales)
        loss_sb = const.tile([BC, 1], F32, name="loss_sb")
        nc.vector.scalar_tensor_tensor(
            out=loss_sb,
            in0=numer_ps,
            scalar=float(L * S + 48.0 * np.log(2.0)),
            in1=logfin,
            op0=mybir.AluOpType.subtract,
            op1=mybir.AluOpType.subtract,
        )
        nc.sync.dma_start(out=dram_ap(loss_t, 0, [[1, BC], [1, 1]]), in_=loss_sb)

    nc.finalize()
    return nc


def host_inputs(inputs, tags, length=L):
    """Per-core slot-relaid inputs (host-side sharding / layout prep only)."""
    inputs = np.asarray(inputs, dtype=np.float32)
    tags = np.asarray(tags)

    nch = len(CHUNK_BOUNDS) - 1
    W0 = CHUNK_BOUNDS[1]
    in_maps = []
    for cc in range(NCORES):
        bsl = slice(cc * BC, (cc + 1) * BC)
        xr = inputs[bsl].reshape(BC, length, JCN, 128)   # (8, 1024, 2, 128)
        em = np.zeros((128, SLOTS, JCN, NCOL), dtype=np.float32)
        em[:, 0:NK, :, 0:BC] = xr[:, 0:NK].transpose(3, 1, 2, 0)
        em[:, 1:NK + 1, :, BC:NCOL] = xr[:, length - 1 : NK - 1 : -1].transpose(3, 1, 2, 0)
        # chunked [jc, c, s_local] layout, each chunk padded to W0 slots
        em_ch = np.zeros((128, nch, JCN, NCOL, W0), dtype=np.float32)
        for ci in range(nch):
            s0, s1 = CHUNK_BOUNDS[ci], CHUNK_BOUNDS[ci + 1]
            em_ch[:, ci, :, :, 0 : s1 - s0] = em[:, s0:s1].transpose(0, 2, 3, 1)
        tg = np.full((SLOTS, NCOL), DUMMY_TAG, dtype=np.float32)
        tg[0:SLOTS, 0:BC] = tags[bsl][:, 0:SLOTS].T
        tg[1:NK + 1, BC:NCOL] = tags[bsl][:, length - 1 : NK - 1 : -1].T
        in_maps.append(
            dict(em=em_ch.reshape(-1, 1), tags_sc=tg.reshape(-1, 1))
        )
    return in_maps


def host_shared(transitions, start_transitions, stop_transitions):
    tr = np.asarray(transitions, dtype=np.float32)
    aux = np.zeros((AUX_N, 1), dtype=np.float32)
    aux[: T * T, 0] = tr.reshape(-1)               # i-major (fwd E tiles)
    aux[T * T : 2 * T * T, 0] = tr.T.reshape(-1)   # j-major (bwd ET tiles)
    aux[AUX_SS : AUX_SS + T, 0] = np.asarray(start_transitions, np.float32)
    aux[AUX_SS + T :, 0] = np.asarray(stop_transitions, np.float32)
    iota = np.arange(128, dtype=np.float32).reshape(128, 1)
    return dict(aux=aux, iota=iota)


def kernel(inputs, tags, mask, transitions, start_transitions, stop_transitions):
    del mask  # all-ones per the problem spec
    in_maps = host_inputs(inputs, tags)
    shared = host_shared(transitions, start_transitions, stop_transitions)
    for m in in_maps:
        m.update(shared)

    nc = build_program()
    res = run_bass_kernel_spmd(nc, in_maps, core_ids=list(range(NCORES)))
    out = np.concatenate([r["loss"].reshape(BC) for r in res.results])
    return out.astype(np.float32)


if __name__ == "__main__":
    rng = np.random.default_rng(0)
    inputs = rng.standard_normal((B, L, T), dtype=np.float32)
    tags = rng.integers(0, T, size=(B, L))
    trans = rng.standard_normal((T, T)).astype(np.float32)
    start = rng.standard_normal(T).astype(np.float32)
    stop = rng.standard_normal(T).astype(np.float32)
    out = kernel(inputs, tags, np.ones((B, L), bool), trans, start, stop)
    print(out)


# revision 25
# speedup vs baseline: 2.0906x; 1.1323x over previous
"""ConditionalRandomField loss kernel for Trainium2 (8 NeuronCores).

Math (per sequence b):
    loss[b] = log_score(gold path) - log_partition

log_partition via a meet-in-the-middle linear scan in exp space:
    fwd:  F_t = (E^T F_{t-1}) * g_t        t = 1..511,  F_0 = exp(start)*g_0
    bwd:  B_t = E (g_t * B_{t+1})          t = 1023..512, B_1024 = exp(stop)
    Z    = sum_j F_511[j] * B_512[j]
with E = exp(transitions) in fp8e4m3 (PE weights) and g_t = exp(emit_t - S)
(S = 6.5 folded shift keeps the running product in bf16 range with no
per-step rescaling; log Z = ln(Z_hat) + 1024*S).  Halves the sequential
depth to 512 steps, and the fwd/bwd chains hide each other's
PE->PSUM->DVE->PE round-trip latency.

Emissions/tags are host-relaid in "slot" order: slot k columns 0-7 hold
t=k (fwd), columns 8-15 hold t=1024-k (bwd), so one sequential DMA feeds
both chains from slot 0 upward and the numerator indexing stays uniform.

The gold-path numerator uses one-hot tag masks (built on device from an
iota compare): emissions via fused multiply-accumulate against the raw
emission chunks, transitions[tag_t, tag_t+1] via y = Tr^T @ OH matmuls
followed by a masked accumulate against the +-1-slot-shifted one-hot,
start/stop via tiny matmuls.

Sharding: data-parallel over batch; core c owns sequences [8c, 8c+8).

NOTE: mask is all-ones for this problem spec (fill: ones); the kernel
assumes it (the reference's masked branches are identities then).
"""

import numpy as np
from contextlib import ExitStack

import concourse.bass as bass
import concourse.bacc as bacc
import concourse.tile as tile
from concourse import mybir
from concourse.bass_utils import run_bass_kernel_spmd

F32 = mybir.dt.float32
BF16 = mybir.dt.bfloat16
FP8 = mybir.dt.float8e4

NCORES = 8
B = 64
L = 1024
T = 256
BC = B // NCORES      # sequences per core
JCN = T // 128        # = 2 tag chunks
NK = L // 2           # scan iterations (fwd+bwd per iteration)
SLOTS = NK + 1        # emission slots (slot k: fwd t=k | bwd t=1024-k)
NCOL = 2 * BC         # 16 columns per slot (fwd 8 | bwd 8)
S = 6.5               # log-shift folded into g = exp(emit - S)
DUMMY_TAG = 999.0     # never matches a one-hot row

AUX_TT = T * T        # aux: [trans i-major | trans j-major | start | stop]
AUX_SS = 2 * T * T
AUX_N = 2 * T * T + 2 * T

CHUNK_BOUNDS = [0, 129, 257, 385, 513]   # slot chunks for the em load


class _Bacc(bacc.Bacc):
    def __init__(self, move_waits=True):
        super().__init__()
        self._move_waits = move_waits

    def move_matmul_waits_to_ldweights(self):
        # Moving extra MM waits onto LDWEIGHTS blocks weight prefetch during
        # the DVE phase; disabled, the framework splits waits via
        # EVENT_SEMAPHORE and the (data-independent) LDW can run early.
        if self._move_waits:
            super().move_matmul_waits_to_ldweights()


def build_program(move_waits=True, debug=False):
    nc = _Bacc(move_waits=move_waits)
    nch_ = len(CHUNK_BOUNDS) - 1
    em_t = nc.declare_dram_parameter(
        "em", [128 * nch_ * 2 * NCOL * CHUNK_BOUNDS[1], 1], F32, isOutput=False
    )
    aux_t = nc.declare_dram_parameter("aux", [AUX_N, 1], F32, isOutput=False)
    tags_t = nc.declare_dram_parameter("tags_sc", [SLOTS * NCOL, 1], F32, isOutput=False)
    iota_t = nc.declare_dram_parameter("iota", [128, 1], F32, isOutput=False)
    loss_t = nc.declare_dram_parameter("loss", [BC, 1], F32, isOutput=True)
    dbg_t = nc.declare_dram_parameter("dbg", [128 * 128, 1], F32, isOutput=True) if debug else None

    def dram_ap(handle, offset, ap):
        full = handle[:]
        return bass.AP(tensor=full.tensor, offset=offset, ap=ap)

    with tile.TileContext(nc) as tc, ExitStack() as ctx:
        const = ctx.enter_context(tc.tile_pool(name="const", bufs=1))
        stage = ctx.enter_context(tc.tile_pool(name="stage", bufs=3))
        tpool = ctx.enter_context(tc.tile_pool(name="tpool", bufs=1))
        gpool = ctx.enter_context(tc.tile_pool(name="gpool", bufs=1))
        fpool = ctx.enter_context(tc.tile_pool(name="fpool", bufs=3))
        upool = ctx.enter_context(tc.tile_pool(name="upool", bufs=3))
        pfpool = ctx.enter_context(tc.tile_pool(name="pfpool", bufs=2, space="PSUM"))
        pbpool = ctx.enter_context(tc.tile_pool(name="pbpool", bufs=2, space="PSUM"))
        ypool = ctx.enter_context(tc.tile_pool(name="ypool", bufs=2, space="PSUM"))
        smallp = ctx.enter_context(tc.tile_pool(name="smallp", bufs=2, space="PSUM"))

        # ---------------- constants ----------------
        iota_sb = const.tile([128, 1], F32, name="iota_sb")
        nc.sync.dma_start(out=iota_sb, in_=iota_t[:])

        neg_shift = const.tile([128, 1], F32, name="neg_shift")
        nc.vector.memset(neg_shift, -S)

        # raw chunks and gbuf are both laid out [jc, c, s] so the exp and
        # the gold-emission gathers read/write contiguously; the scan's
        # per-k multiply reads 16 strided elements instead (negligible).
        # Chunk 0 is DMA'd before everything else so the scan starts early.
        gbuf = gpool.tile([128, JCN, NCOL, SLOTS], BF16, name="gbuf")
        W0 = CHUNK_BOUNDS[1]
        nch = len(CHUNK_BOUNDS) - 1
        row = nch * 2 * NCOL * W0
        raw_tiles = {}

        def load_chunk(ci):
            s0, s1 = CHUNK_BOUNDS[ci], CHUNK_BOUNDS[ci + 1]
            w = s1 - s0
            raw = stage.tile([128, JCN, NCOL, W0], F32, name="raw", tag="raw")
            nc.sync.dma_start(
                out=raw,
                in_=dram_ap(
                    em_t, ci * 2 * NCOL * W0, [[row, 128], [1, 2 * NCOL * W0]]
                ),
            )
            gb_out = bass.AP(
                tensor=gbuf.tensor,
                offset=gbuf.offset + s0,
                ap=[gbuf.ap[0], [NCOL * SLOTS, JCN], [SLOTS, NCOL], [1, w]],
            )
            raw_in = bass.AP(
                tensor=raw.tensor,
                offset=raw.offset,
                ap=[raw.ap[0], [W0 * NCOL, JCN], [W0, NCOL], [1, w]],
            )
            nc.scalar.activation(
                out=gb_out,
                in_=raw_in,
                func=mybir.ActivationFunctionType.Exp,
                bias=neg_shift[:],
            )
            raw_tiles[ci] = raw

        load_chunk(0)

        # E tiles: exp(trans) fp8, i-chunk major; TR tiles: raw trans bf16.
        e_tiles, tr_tiles = [], []
        for ic in range(JCN):
            eraw = stage.tile([128, T], F32, name=f"eraw{ic}", tag="eraw")
            nc.sync.dma_start(
                out=eraw, in_=dram_ap(aux_t, ic * 128 * T, [[T, 128], [1, T]])
            )
            ebf = const.tile([128, T], FP8, name=f"ebf{ic}")
            nc.scalar.activation(out=ebf, in_=eraw, func=mybir.ActivationFunctionType.Exp)
            e_tiles.append(ebf)
            trbf = const.tile([128, T], BF16, name=f"trbf{ic}")
            nc.vector.tensor_copy(out=trbf, in_=eraw)
            tr_tiles.append(trbf)
        # ET tiles: exp(trans)^T fp8, j-chunk major (for the bwd chain).
        et_tiles = []
        for jc in range(JCN):
            eraw = stage.tile([128, T], F32, name=f"etraw{jc}", tag="eraw")
            nc.sync.dma_start(
                out=eraw,
                in_=dram_ap(aux_t, AUX_TT + jc * 128 * T, [[T, 128], [1, T]]),
            )
            etbf = const.tile([128, T], FP8, name=f"etbf{jc}")
            nc.scalar.activation(out=etbf, in_=eraw, func=mybir.ActivationFunctionType.Exp)
            et_tiles.append(etbf)

        # start/stop: raw bf16 (numerator) + exp f32 (scan boundary values)
        ssraw = stage.tile([128, 2 * JCN], F32, name="ssraw", tag="eraw")
        nc.sync.dma_start(
            out=ssraw[:, 0:JCN], in_=dram_ap(aux_t, AUX_SS, [[1, 128], [128, JCN]])
        )
        nc.sync.dma_start(
            out=ssraw[:, JCN : 2 * JCN],
            in_=dram_ap(aux_t, AUX_SS + T, [[1, 128], [128, JCN]]),
        )
        ssbf = const.tile([128, 2 * JCN], BF16, name="ssbf")
        nc.vector.tensor_copy(out=ssbf, in_=ssraw)
        sstart = const.tile([128, JCN], F32, name="sstart")
        nc.scalar.activation(
            out=sstart, in_=ssraw[:, 0:JCN], func=mybir.ActivationFunctionType.Exp
        )
        sstop = const.tile([128, JCN], F32, name="sstop")
        nc.scalar.activation(
            out=sstop, in_=ssraw[:, JCN : 2 * JCN], func=mybir.ActivationFunctionType.Exp
        )
        ones8 = const.tile([128, BC], BF16, name="ones8")
        nc.vector.memset(ones8, 1.0)
        ones_col = const.tile([128, 1], BF16, name="ones_col")
        nc.vector.memset(ones_col, 1.0)
        ones_col_f = const.tile([128, 1], F32, name="ones_col_f")
        nc.vector.memset(ones_col_f, 1.0)
        # B_1024 = exp(stop) replicated over the 8 bwd columns
        bstop = const.tile([128, JCN, BC], BF16, name="bstop")
        for jc in range(JCN):
            nc.vector.tensor_scalar_mul(
                out=bstop[:, jc, :], in0=ones8, scalar1=sstop[:, jc : jc + 1]
            )

        # ---------------- one-hot masks (built chunked, inside the scan) --
        # OH_jc[p, s*16+c] = 1.0 iff tags_sc[s, c] == jc*128 + p
        tags_bc = tpool.tile([128, SLOTS * NCOL], F32, name="tags_bc")
        nc.sync.dma_start(
            out=tags_bc, in_=dram_ap(tags_t, 0, [[0, 128], [1, SLOTS * NCOL]])
        )
        oh_tiles = [
            const.tile([128, SLOTS * NCOL], BF16, name=f"oh{jc}") for jc in range(JCN)
        ]

        def build_oh_piece(jc, p0, p1):
            nc.vector.tensor_scalar(
                out=oh_tiles[jc][:, p0:p1],
                in0=tags_bc[:, p0:p1],
                scalar1=float(jc * 128),
                scalar2=iota_sb[:],
                op0=mybir.AluOpType.subtract,
                op1=mybir.AluOpType.is_equal,
            )

        # ---------------- emissions: load + exp (gathers run in-scan) -----
        nch = len(CHUNK_BOUNDS) - 1
        acc2e = const.tile([128, nch * 2 * NCOL], F32, name="acc2e")
        acc2t = const.tile([128, 2 * NCOL], F32, name="acc2t")
        scr_g = const.tile([128, CHUNK_BOUNDS[1]], BF16, name="scr_g")
        scr_v = const.tile([128, NK], BF16, name="scr_v")

        for ci in range(1, nch):
            load_chunk(ci)

        def g_slice(k, c0, c1):
            # [128, JCN, c1-c0] view of g at slot k (strided over c)
            return bass.AP(
                tensor=gbuf.tensor,
                offset=gbuf.offset + c0 * SLOTS + k,
                ap=[gbuf.ap[0], [NCOL * SLOTS, JCN], [SLOTS, c1 - c0]],
            )

        def emit_gather(ci, jc, c):
            # gold emission: acc += sum_s raw[p, jc, c, s] * OH[p, s*16+c]
            # fwd cols use slots 0..511, bwd cols slots 1..512 (exact cover).
            s0, s1 = CHUNK_BOUNDS[ci], CHUNK_BOUNDS[ci + 1]
            a = max(s0, 1) if c >= BC else s0
            b_ = s1 if c >= BC else min(s1, NK)
            n = b_ - a
            if n <= 0:
                return
            acol = (ci * 2 + jc) * NCOL + c
            nc.vector.scalar_tensor_tensor(
                out=scr_g[:, 0:n],
                in0=raw_tiles[ci][:, jc, c, a - s0 : b_ - s0],
                scalar=1.0,
                in1=oh_tiles[jc][:, c * SLOTS + a : c * SLOTS + b_],
                op0=mybir.AluOpType.mult,
                op1=mybir.AluOpType.mult,
                accum_out=acc2e[:, acol : acol + 1],
            )

        # DVE side-work schedule, paced so no piece exceeds the per-k DVE
        # idle window: one-hot pieces (~129 cols) 1/k over k=1..~128 in chunk
        # order, then each chunk's 32 gathers 1/k once its raw tile + OH
        # chunk exist.
        side_work = {}
        kq = 1
        for jc in range(JCN):
            for c in range(NCOL):
                for p0, p1 in ((0, 257), (257, SLOTS)):
                    side_work.setdefault(kq, []).append(
                        ("oh", jc, c * SLOTS + p0, c * SLOTS + p1)
                    )
                    kq += 1
        for ci in range(nch):
            kg = max(kq + 1, 130 + 33 * ci)
            for jc in range(JCN):
                for c in range(NCOL):
                    side_work.setdefault(kg, []).append(("gather", ci, jc, c))
                    kg += 1

        # ---------------- the scan ----------------
        fw = fpool.tile([128, JCN, BC], BF16, name="fw", tag="fw")
        for jc in range(JCN):
            nc.vector.tensor_scalar_mul(
                out=fw[:, jc, :],
                in0=bass.AP(
                    tensor=gbuf.tensor,
                    offset=gbuf.offset + jc * NCOL * SLOTS,
                    ap=[gbuf.ap[0], [SLOTS, BC]],
                ),
                scalar1=sstart[:, jc : jc + 1],
            )

        def dbg_dump(col, tile_in, n=NCOL):
            if dbg_t is None:
                return
            d = const.tile([128, n], F32, name=f"dbg{col}")
            nc.vector.tensor_copy(out=d, in_=tile_in)
            nc.sync.dma_start(
                out=dram_ap(dbg_t, col, [[128, 128], [1, n]]), in_=d
            )

        if debug:
            dbg_dump(0, g_slice(1, 0, BC))
            dbg_dump(16, g_slice(1, BC, NCOL))
            dbg_dump(32, g_slice(256, 0, BC))
            dbg_dump(48, g_slice(512, BC, NCOL))
            dbg_dump(64, fw)

        pb = None
        pf = None
        fw_pend = None   # fw(k-1) rhs for the pending fwd group

        def emit_side(k):
            for work in side_work.get(k, ()):
                if work[0] == "oh":
                    build_oh_piece(work[1], work[2], work[3])
                else:
                    emit_gather(work[1], work[2], work[3])

        def emit_fwd_group(rhs):
            p = pfpool.tile([128, JCN, BC], F32, name="pf", tag="pf")
            nc.tensor.matmul(out=p[:, 0, :], lhsT=e_tiles[0][:, 0:128], rhs=rhs[:, 0, :], start=True, stop=False)
            nc.tensor.matmul(out=p[:, 0, :], lhsT=e_tiles[1][:, 0:128], rhs=rhs[:, 1, :], start=False, stop=True)
            nc.tensor.matmul(out=p[:, 1, :], lhsT=e_tiles[0][:, 128:256], rhs=rhs[:, 0, :], start=True, stop=False)
            nc.tensor.matmul(out=p[:, 1, :], lhsT=e_tiles[1][:, 128:256], rhs=rhs[:, 1, :], start=False, stop=True)
            return p

        # skewed pipeline: per iteration k emit
        #   [PE fwd_group(k-1)] [DVE mult_b(k)] [PE bwd_group(k)] [DVE mult_f(k-1)]
        # so each PE group has exactly one mult+drain ahead of it, and the
        # two DVE mults never sit back-to-back on the critical path.
        for k in range(1, NK + 1):
            emit_side(k)
            if k >= 2:
                pf = emit_fwd_group(fw)

            u = upool.tile([128, JCN, BC], BF16, name="u", tag="u")
            nc.vector.tensor_tensor(
                out=u,
                in0=(bstop if k == 1 else pb),
                in1=g_slice(k, BC, NCOL),
                op=mybir.AluOpType.mult,
            )
            if k == NK // 2:
                # one mid-scan 2^-24 rescale per chain keeps the final dot
                # product inside the ACT Ln table range (breaks above ~1e17)
                us = upool.tile([128, JCN, BC], BF16, name="u", tag="u")
                nc.vector.tensor_scalar_mul(out=us, in0=u, scalar1=2.0 ** -24)
                u = us
            if debug and k in (2, 64, 256, 400):
                dbg_dump({2: 80, 64: 84, 256: 88, 400: 92}[k], u[:, :, 0:2], 4)

            pb = pbpool.tile([128, JCN, BC], F32, name="pb", tag="pb")
            nc.tensor.matmul(out=pb[:, 0, :], lhsT=et_tiles[0][:, 0:128], rhs=u[:, 0, :], start=True, stop=False)
            nc.tensor.matmul(out=pb[:, 0, :], lhsT=et_tiles[1][:, 0:128], rhs=u[:, 1, :], start=False, stop=True)
            nc.tensor.matmul(out=pb[:, 1, :], lhsT=et_tiles[0][:, 128:256], rhs=u[:, 0, :], start=True, stop=False)
            nc.tensor.matmul(out=pb[:, 1, :], lhsT=et_tiles[1][:, 128:256], rhs=u[:, 1, :], start=False, stop=True)

            if k >= 2:
                j = k - 1
                fw2 = fpool.tile([128, JCN, BC], BF16, name="fw", tag="fw")
                nc.vector.tensor_tensor(
                    out=fw2, in0=pf, in1=g_slice(j, 0, BC), op=mybir.AluOpType.mult
                )
                fw = fw2
                if debug and j in (2, 64, 256, 400):
                    dbg_dump({2: 96, 64: 100, 256: 104, 400: 108}[j], fw[:, :, 0:2], 4)
                if j == NK // 2:
                    fws = fpool.tile([128, JCN, BC], BF16, name="fw", tag="fw")
                    nc.vector.tensor_scalar_mul(out=fws, in0=fw, scalar1=2.0 ** -24)
                    fw = fws

        # ---------------- gold transition scores ----------------
        # y[j', s] = Tr[tag_s, j']; acc += sum_s y[j', s] * OH_{s+-1}[j', s]
        for side in range(2):
            for b in range(BC):
                c = side * BC + b
                if side == 0:
                    sa, n, shift = 0, NK, 1        # slots 0..511, next t at +1 slot
                else:
                    sa, n, shift = 2, NK - 1, -1   # slots 2..512, next t at -1 slot
                base = c * SLOTS + sa
                for jcp in range(JCN):
                    y_ps = ypool.tile([128, NK], F32, name="y_ps", tag="y")
                    for ic in range(JCN):
                        nc.tensor.matmul(
                            out=y_ps[:, 0:n],
                            lhsT=tr_tiles[ic][:, jcp * 128 : (jcp + 1) * 128],
                            rhs=oh_tiles[ic][:, base : base + n],
                            start=(ic == 0),
                            stop=(ic == JCN - 1),
                        )
                    acol = jcp * NCOL + c
                    nc.vector.scalar_tensor_tensor(
                        out=scr_v[:, 0:n],
                        in0=y_ps[:, 0:n],
                        scalar=1.0,
                        in1=oh_tiles[jcp][:, base + shift : base + shift + n],
                        op0=mybir.AluOpType.mult,
                        op1=mybir.AluOpType.mult,
                        accum_out=acc2t[:, acol : acol + 1],
                    )

        # ---------------- finalization ----------------
        # Z = sum_j F_511 * B_512 (per column pair)
        h = const.tile([128, JCN, BC], BF16, name="h")
        nc.vector.tensor_tensor(out=h, in0=pb, in1=fw, op=mybir.AluOpType.mult)
        if debug:
            dbg_dump(112, h)
        s4 = smallp.tile([BC, 1], F32, name="s4", tag="small")
        nc.tensor.matmul(out=s4, lhsT=h[:, 0, :], rhs=ones_col, start=True, stop=False)
        nc.tensor.matmul(out=s4, lhsT=h[:, 1, :], rhs=ones_col, start=False, stop=True)
        logfin = const.tile([BC, 1], F32, name="logfin")
        nc.scalar.activation(out=logfin, in_=s4, func=mybir.ActivationFunctionType.Ln)

        # fold per-call accumulators: rede/redt [128, 16] col c
        rede = const.tile([128, NCOL], F32, name="rede")
        e_view = bass.AP(
            tensor=acc2e.tensor,
            offset=acc2e.offset,
            ap=[acc2e.ap[0], [1, NCOL], [NCOL, nch * 2]],
        )
        nc.vector.tensor_reduce(
            out=rede, in_=e_view, axis=mybir.AxisListType.X, op=mybir.AluOpType.add
        )
        redt = const.tile([128, NCOL], F32, name="redt")
        t_view = bass.AP(
            tensor=acc2t.tensor,
            offset=acc2t.offset,
            ap=[acc2t.ap[0], [1, NCOL], [NCOL, 2]],
        )
        nc.vector.tensor_reduce(
            out=redt, in_=t_view, axis=mybir.AxisListType.X, op=mybir.AluOpType.add
        )
        numacc = const.tile([128, NCOL], F32, name="numacc")
        nc.vector.tensor_add(out=numacc, in0=rede, in1=redt)

        # numer[b] = sum_p numacc[p, b] + numacc[p, 8+b]
        #          + start[tag_0] + stop[tag_1023]
        numer_ps = smallp.tile([BC, 1], F32, name="numer_ps", tag="small")
        nc.tensor.matmul(out=numer_ps, lhsT=numacc[:, 0:BC], rhs=ones_col_f, start=True, stop=False)
        nc.tensor.matmul(out=numer_ps, lhsT=numacc[:, BC:NCOL], rhs=ones_col_f, start=False, stop=False)
        def oh_col_view(jc, c0, s):
            t = oh_tiles[jc]
            return bass.AP(
                tensor=t.tensor,
                offset=t.offset + c0 * SLOTS + s,
                ap=[t.ap[0], [SLOTS, BC]],
            )
        nc.tensor.matmul(out=numer_ps, lhsT=oh_col_view(0, 0, 0), rhs=ssbf[:, 0:1], start=False, stop=False)
        nc.tensor.matmul(out=numer_ps, lhsT=oh_col_view(1, 0, 0), rhs=ssbf[:, 1:2], start=False, stop=False)
        nc.tensor.matmul(out=numer_ps, lhsT=oh_col_view(0, BC, 1), rhs=ssbf[:, 2:3], start=False, stop=False)
        nc.tensor.matmul(out=numer_ps, lhsT=oh_col_view(1, BC, 1), rhs=ssbf[:, 3:4], start=False, stop=True)

        # loss = (numer - L*S - 48*ln2) - ln(Z_hat)   (2^-48 from the rescales)
        loss_sb = const.tile([BC, 1], F32, name="loss_sb")
        nc.vector.scalar_tensor_tensor(
            out=loss_sb,
            in0=numer_ps,
            scalar=float(L * S + 48.0 * np.log(2.0)),
            in1=logfin,
            op0=mybir.AluOpType.subtract,
            op1=mybir.AluOpType.subtract,
        )
        nc.sync.dma_start(out=dram_ap(loss_t, 0, [[1, BC], [1, 1]]), in_=loss_sb)

    nc.finalize()
    return nc


def host_inputs(inputs, tags, length=L):
    """Per-core slot-relaid inputs (host-side sharding / layout prep only)."""
    inputs = np.asarray(inputs, dtype=np.float32)
    tags = np.asarray(tags)

    nch = len(CHUNK_BOUNDS) - 1
    W0 = CHUNK_BOUNDS[1]
    in_maps = []
    for cc in range(NCORES):
        bsl = slice(cc * BC, (cc + 1) * BC)
        xr = inputs[bsl].reshape(BC, length, JCN, 128)   # (8, 1024, 2, 128)
        em = np.zeros((128, SLOTS, JCN, NCOL), dtype=np.float32)
        em[:, 0:NK, :, 0:BC] = xr[:, 0:NK].transpose(3, 1, 2, 0)
        em[:, 1:NK + 1, :, BC:NCOL] = xr[:, length - 1 : NK - 1 : -1].transpose(3, 1, 2, 0)
        # chunked [jc, c, s_local] layout, each chunk padded to W0 slots
        em_ch = np.zeros((128, nch, JCN, NCOL, W0), dtype=np.float32)
        for ci in range(nch):
            s0, s1 = CHUNK_BOUNDS[ci], CHUNK_BOUNDS[ci + 1]
            em_ch[:, ci, :, :, 0 : s1 - s0] = em[:, s0:s1].transpose(0, 2, 3, 1)
        tg = np.full((NCOL, SLOTS), DUMMY_TAG, dtype=np.float32)
        tg[0:BC, 0:SLOTS] = tags[bsl][:, 0:SLOTS]
        tg[BC:NCOL, 1:NK + 1] = tags[bsl][:, length - 1 : NK - 1 : -1]
        in_maps.append(
            dict(em=em_ch.reshape(-1, 1), tags_sc=tg.reshape(-1, 1))
        )
    return in_maps


def host_shared(transitions, start_transitions, stop_transitions):
    tr = np.asarray(transitions, dtype=np.float32)
    aux = np.zeros((AUX_N, 1), dtype=np.float32)
    aux[: T * T, 0] = tr.reshape(-1)               # i-major (fwd E tiles)
    aux[T * T : 2 * T * T, 0] = tr.T.reshape(-1)   # j-major (bwd ET tiles)
    aux[AUX_SS : AUX_SS + T, 0] = np.asarray(start_transitions, np.float32)
    aux[AUX_SS + T :, 0] = np.asarray(stop_transitions, np.float32)
    iota = np.arange(128, dtype=np.float32).reshape(128, 1)
    return dict(aux=aux, iota=iota)


def kernel(inputs, tags, mask, transitions, start_transitions, stop_transitions):
    del mask  # all-ones per the problem spec
    in_maps = host_inputs(inputs, tags)
    shared = host_shared(transitions, start_transitions, stop_transitions)
    for m in in_maps:
        m.update(shared)

    nc = build_program()
    res = run_bass_kernel_spmd(nc, in_maps, core_ids=list(range(NCORES)))
    out = np.concatenate([r["loss"].reshape(BC) for r in res.results])
    return out.astype(np.float32)


if __name__ == "__main__":
    rng = np.random.default_rng(0)
    inputs = rng.standard_normal((B, L, T), dtype=np.float32)
    tags = rng.integers(0, T, size=(B, L))
    trans = rng.standard_normal((T, T)).astype(np.float32)
    start = rng.standard_normal(T).astype(np.float32)
    stop = rng.standard_normal(T).astype(np.float32)
    out = kernel(inputs, tags, np.ones((B, L), bool), trans, start, stop)
    print(out)
